# revision 20
# baseline (speedup 1.0000x reference)
"""Trainium2 Bass kernel for GQA attention (B=2,S=2048,D=2048,H=16,KV=4,HD=128)
with RoPE + causal mask, sharded over 8 NeuronCores:
  2-way data parallel over batch x 4-way tensor parallel over KV groups.

Core c = (b, g): b = c // 4, g = c % 4.
Each core computes, for its batch b and KV group g (q heads 4g..4g+3):
  QT_h [HD,S], KT [HD,S] (RoPE'd), V [S,HD]    via matmul vs xT [D,S]
  scoresT [sk,sq] blocks, exp on ScalarE (scale folded), row-sums via an
  all-ones matmul (which also replicates the sums across partitions),
  AV with V tiles stationary -> outT [HD,sq], per-head normalization via
  reciprocal, partial y = attn_norm @ wo_rows[g]; host sums the 4 partials.

matmul(out, lhsT, rhs) = lhsT.T @ rhs, contraction over the partition dim.
All contractions are K=128.  Causality at block granularity: fully-masked
(sk,sq) blocks skipped; diagonal blocks add the mask slice (pattern repeats
every 4 sk-tiles, so only a [512,512] mask transpose is shipped).

Matmul operands are bf16 (full-kernel relative error ~3e-3 vs the fp32
reference; the gate is 2e-2).  All accumulation is fp32 in PSUM; softmax
exp/normalization arithmetic is fp32.  The output projection for sq-chunk c
is software-pipelined one chunk behind attention so its PE work fills the
softmax-dependency bubbles of chunk c+1, and y tiles are DMA'd straight
from PSUM.
"""

import os
from contextlib import ExitStack

import numpy as np
import ml_dtypes

import concourse.bacc as bacc
import concourse.mybir as mybir
import concourse.tile as tile

# ---------------- problem constants (hardcoded per contract) ----------------
B, S, D = 2, 2048, 2048
H, KV, HD = 16, 4, 128
REP = H // KV            # 4 q heads per kv head
NG = KV                  # 4 tensor-parallel groups
NCORES = 8
THETA = 10000.0
SCALE = 1.0 / float(np.sqrt(HD))

P = 128                  # partition dim
SC = 512                 # moving free-dim chunk (one PSUM bank in fp32)
NDT = S // P             # 16 tiles of 128 along S or D
NCH = S // SC            # 4 chunks of 512 along S
NH = REP                 # 4 q-heads per core

FP32 = mybir.dt.float32
BF16 = mybir.dt.bfloat16

# matmul dtype: "bf16" (measured full-kernel relative error ~3e-3 vs the
# fp32 reference -- gate is 2e-2) or "fp32" (exact to ~1e-6, 4 cyc/row).
MM_MODE = os.environ.get("KERNEL_MM_MODE", "bf16")

_CACHE = {}


def _build_program(mm_mode=MM_MODE, repeat=1):
    MDT = BF16 if mm_mode == "bf16" else FP32

    nc = bacc.Bacc("TRN2", target_bir_lowering=False, debug=False)

    xT_d = nc.dram_tensor("xT", [D, S], MDT, kind="ExternalInput").ap()
    wq_d = nc.dram_tensor("wqg", [D, NH * HD], MDT, kind="ExternalInput").ap()
    wk_d = nc.dram_tensor("wkg", [D, HD], MDT, kind="ExternalInput").ap()
    wv_d = nc.dram_tensor("wvg", [D, HD], MDT, kind="ExternalInput").ap()
    wo_d = nc.dram_tensor("wog", [NH * HD, D], MDT, kind="ExternalInput").ap()
    cosT_d = nc.dram_tensor("cosT", [HD, S], FP32, kind="ExternalInput").ap()
    sinrT_d = nc.dram_tensor("sinrotT", [HD, S], FP32, kind="ExternalInput").ap()
    mdiag_d = nc.dram_tensor("maskdiag", [NCH * P, SC], FP32, kind="ExternalInput").ap()
    ident_d = nc.dram_tensor("ident", [P, P], FP32, kind="ExternalInput").ap()
    ones_d = nc.dram_tensor("ones", [P, P], MDT, kind="ExternalInput").ap()
    y_d = nc.dram_tensor("y", [S, D], MDT, kind="ExternalOutput").ap()

    with tile.TileContext(nc) as tc, ExitStack() as ctx:
        qkv = ctx.enter_context(tc.tile_pool(name="qkv", bufs=1))
        small = ctx.enter_context(tc.tile_pool(name="small", bufs=1))

        # resident Q^T per head, K^T, V tiles
        qt = [qkv.tile([P, S], MDT, tag=f"qt{h}", name=f"qt{h}") for h in range(NH)]
        kt = qkv.tile([P, S], MDT, tag="kt", name="kt")
        v_tiles = [qkv.tile([P, HD], MDT, tag=f"v{k}", name=f"v{k}")
                   for k in range(NDT)]

        ones_sb = small.tile([P, P], MDT, tag="ones")
        ident_sb = small.tile([P, P], FP32, tag="ident")
        mdiag_slab = small.tile([P, NCH * SC], FP32, tag="mds")
        mdiag_sb = [mdiag_slab[:, r * SC:(r + 1) * SC] for r in range(NCH)]
        wos = small.tile([P, NH * D], MDT, tag="wos")
        wo_sb = [wos[:, h * D:(h + 1) * D] for h in range(NH)]

        def load_consts():
            nc.gpsimd.dma_start(ones_sb[:], ones_d[:])
            nc.gpsimd.dma_start(ident_sb[:], ident_d[:])
            nc.gpsimd.dma_start(
                mdiag_slab[:].rearrange("p (r s) -> p r s", r=NCH),
                mdiag_d.rearrange("(r p) s -> p r s", p=P))

        for rep in range(repeat):
            # ============== phase 1: QKV projection + RoPE ==============
            with tc.tile_pool(name="p1", bufs=1) as p1, \
                 tc.tile_pool(name="xin", bufs=8) as xin, \
                 tc.tile_pool(name="rtmp", bufs=3) as rtmp, \
                 tc.tile_pool(name="ps1", bufs=2, space="PSUM") as ps1:

                # weight slab, head-major: head m's d-tile k lives at slab
                # columns [m*NDT*HD + k*HD, ...).  One DMA per head, head 0
                # first, so the first matmul chain is gated by x-chunk0 +
                # head-0 weights only (~2.5 MB in bf16).
                XQ = NDT // 4   # 4 d-tiles per quarter x slab
                wqs = p1.tile([P, NH * NDT * HD], MDT, tag="wqs")
                wks = p1.tile([P, NDT * HD], MDT, tag="wks")
                wvs = p1.tile([P, NDT * HD], MDT, tag="wvs")
                cosT_sb = p1.tile([HD, S], FP32, tag="cosT")
                sinrT_sb = p1.tile([HD, S], FP32, tag="sinrT")
                vT = p1.tile([HD, S], FP32, tag="vT")

                def load_wq_head(m):
                    nc.scalar.dma_start(
                        wqs[:, m * NDT * HD:(m + 1) * NDT * HD]
                        .rearrange("p (n q) -> p n q", n=NDT),
                        wq_d[:, m * HD:(m + 1) * HD]
                        .rearrange("(n p) q -> p n q", p=P))

                if rep == 0:
                    # warm the PE p-state with dummy matmuls on a memset
                    # scratch tile while the first input DMAs are in flight
                    wsrc = p1.tile([P, P], MDT, tag="wsrc")
                    nc.gpsimd.memset(wsrc[:], 0.0)
                    for i in range(48):
                        wps = ps1.tile([P, SC], FP32, tag="proj", bufs=4)
                        nc.tensor.matmul(wps[:, 0:P], wsrc[:], wsrc[:],
                                         start=True, stop=True)
                load_wq_head(0)

                for sc in range(NCH):
                    s0 = sc * SC
                    xq_slabs = []
                    for qq in range(4):
                        xs = xin.tile([P, XQ * SC], MDT, tag="x")
                        nc.sync.dma_start(
                            xs[:].rearrange("p (n s) -> p n s", n=XQ),
                            xT_d[qq * XQ * P:(qq + 1) * XQ * P, s0:s0 + SC]
                            .rearrange("(n p) s -> p n s", p=P))
                        xq_slabs.append(xs)
                    if sc == 0:
                        # phase-1/2 weights + tables: issue after chunk-0 x
                        # so the first chain starts asap; RoPE tables first
                        # (needed by m=0's RoPE), then the later weights
                        nc.gpsimd.dma_start(cosT_sb[:], cosT_d[:])
                        nc.gpsimd.dma_start(sinrT_sb[:], sinrT_d[:])
                        for m in range(1, NH):
                            load_wq_head(m)
                        nc.gpsimd.dma_start(
                            wks[:].rearrange("p (n m) -> p n m", n=NDT),
                            wk_d.rearrange("(n p) m -> p n m", p=P))
                        nc.gpsimd.dma_start(
                            wvs[:].rearrange("p (n m) -> p n m", n=NDT),
                            wv_d.rearrange("(n p) m -> p n m", p=P))
                        if rep == 0:
                            load_consts()
                        nc.gpsimd.dma_start(
                            wos[:].rearrange("p (n d) -> p n d", n=NH),
                            wo_d.rearrange("(n p) d -> p n d", p=P))

                    def xts_k(k):
                        return xq_slabs[k // XQ][:, (k % XQ) * SC:(k % XQ + 1) * SC]

                    # m = 0..3: q heads; 4: k; 5: v
                    for m in range(NH + 2):
                        psum = ps1.tile([P, SC], FP32, tag="proj", bufs=4)
                        for k in range(NDT):
                            if m < NH:
                                lhsT = wqs[:, m * NDT * HD + k * HD:
                                           m * NDT * HD + (k + 1) * HD]
                            elif m == NH:
                                lhsT = wks[:, k * HD:(k + 1) * HD]
                            else:
                                lhsT = wvs[:, k * HD:(k + 1) * HD]
                            nc.tensor.matmul(
                                psum[:], lhsT, xts_k(k),
                                start=(k == 0), stop=(k == NDT - 1),
                            )
                        if m <= NH:
                            # RoPE: dst = psum*cosT + shift(psum)*sinrotT
                            dst = (qt[m] if m < NH else kt)[:, s0:s0 + SC]
                            t0 = rtmp.tile([P, SC], FP32, tag="t0")
                            t1 = rtmp.tile([P, SC], FP32, tag="t1")
                            nc.vector.tensor_mul(
                                t0[:], psum[:], cosT_sb[:, s0:s0 + SC])
                            nc.vector.tensor_mul(
                                t1[0:64, :], psum[64:128, :],
                                sinrT_sb[0:64, s0:s0 + SC])
                            nc.vector.tensor_mul(
                                t1[64:128, :], psum[0:64, :],
                                sinrT_sb[64:128, s0:s0 + SC])
                            nc.vector.tensor_add(dst, t0[:], t1[:])
                        else:
                            nc.vector.tensor_copy(vT[:, s0:s0 + SC], psum[:])

                    # transpose this chunk of V^T -> V tiles [S_k=128, HD]
                    for kk in range(SC // P):
                        k = sc * (SC // P) + kk
                        ps_t = ps1.tile([P, P], FP32, tag="vt")
                        nc.tensor.transpose(
                            ps_t[:], vT[:, k * P:(k + 1) * P], ident_sb[:])
                        nc.vector.tensor_copy(v_tiles[k][:], ps_t[:])

            # ========== phase 2: attention + output projection ==========
            # The out-projection for chunk c-1 is emitted interleaved into
            # chunk c's attention (one t-group of 16 matmuls after each
            # head) so the PE never waits on the softmax/normalize chains.
            with tc.tile_pool(name="p2", bufs=2) as p2, \
                 tc.tile_pool(name="pt", bufs=24) as ptp, \
                 tc.tile_pool(name="nrm", bufs=4) as nrm, \
                 tc.tile_pool(name="yst", bufs=4) as yst, \
                 tc.tile_pool(name="ps2", bufs=2, space="PSUM") as ps2, \
                 tc.tile_pool(name="pss", bufs=2, space="PSUM") as pss:

                prev_outT = None

                def emit_outproj_tgroup(outT_tiles, c_prev, t):
                    q0p = c_prev * SC
                    for dci in range(NCH):
                        d0 = dci * SC
                        y_ps = ps2.tile([P, SC], FP32, tag="y", bufs=2)
                        for h in range(NH):
                            nc.tensor.matmul(
                                y_ps[:],
                                outT_tiles[h][:, t * P:(t + 1) * P],
                                wo_sb[h][:, d0:d0 + SC],
                                start=(h == 0), stop=(h == NH - 1),
                            )
                        y_sb = yst.tile([P, SC], MDT, tag="ysb")
                        # alternate the PSUM->SBUF copy between the two
                        # element-wise engines; Act is exp-saturated
                        if dci % 2 == 0:
                            nc.vector.tensor_copy(y_sb[:], y_ps[:])
                        else:
                            nc.scalar.activation(
                                y_sb[:], y_ps[:],
                                mybir.ActivationFunctionType.Copy)
                        row0 = q0p + t * P
                        nc.sync.dma_start(
                            y_d[row0:row0 + P, d0:d0 + SC], y_sb[:])

                for c in range(NCH):
                    q0 = c * SC
                    nk = 4 * c + 4          # active sk tiles (causal)
                    outT = [p2.tile([P, SC], MDT, tag=f"ot{h}",
                                    name=f"ot{h}") for h in range(NH)]
                    for h in range(NH):
                        ptm = {}             # k -> (pt tile, off)
                        red = []             # pre-reduced tiles for row sums
                        ks = list(range(nk))
                        for k in ks:
                            # diagonal blocks: sk tile k only attends to
                            # sq >= 128k, i.e. chunk columns [off:512); only
                            # the leading 128 columns of that are a partial
                            # (triangular) mask -- the rest is fully allowed.
                            off = max(0, (k - 4 * c) * P)
                            sc_ps = ps2.tile([P, SC], FP32, tag="sc", bufs=3)
                            nc.tensor.matmul(
                                sc_ps[:, off:],
                                kt[:, k * P:(k + 1) * P],
                                qt[h][:, q0 + off:q0 + SC],
                                start=True, stop=True,
                            )
                            pt = ptp.tile([P, SC], MDT, tag="pt")
                            if k >= 4 * c:
                                # triangle columns [off:off+128): scale+mask
                                # on DVE then exp; columns beyond are plain
                                r = k % NCH
                                nc.vector.scalar_tensor_tensor(
                                    sc_ps[:, off:off + P], sc_ps[:, off:off + P],
                                    SCALE, mdiag_sb[r][:, off:off + P],
                                    op0=mybir.AluOpType.mult,
                                    op1=mybir.AluOpType.add)
                                nc.scalar.activation(
                                    pt[:, off:off + P], sc_ps[:, off:off + P],
                                    mybir.ActivationFunctionType.Exp)
                                if off + P < SC:
                                    nc.scalar.activation(
                                        pt[:, off + P:], sc_ps[:, off + P:],
                                        mybir.ActivationFunctionType.Exp,
                                        scale=SCALE)
                            else:
                                nc.scalar.activation(
                                    pt[:, off:], sc_ps[:, off:],
                                    mybir.ActivationFunctionType.Exp,
                                    scale=SCALE)
                            ptm[k] = (pt, off)
                            if k == 4 * c + 3:
                                # staircase-sum the 4 diagonal prob tiles
                                d0 = ptm[4 * c][0]
                                d1 = ptm[4 * c + 1][0]
                                d2 = ptm[4 * c + 2][0]
                                d3 = ptm[4 * c + 3][0]
                                ds = ptp.tile([P, SC], MDT, tag="ds", bufs=4)
                                nc.vector.tensor_copy(ds[:, 0:P], d0[:, 0:P])
                                nc.vector.tensor_add(
                                    ds[:, P:], d0[:, P:], d1[:, P:])
                                nc.vector.tensor_add(
                                    ds[:, 2 * P:], ds[:, 2 * P:], d2[:, 2 * P:])
                                nc.vector.tensor_add(
                                    ds[:, 3 * P:], ds[:, 3 * P:], d3[:, 3 * P:])
                                red.append(ds)
                            if k % 4 == 3 and k < 4 * c:
                                # quad-reduce 4 full off-diagonal prob tiles
                                # (alternating DVE / gpsimd) so the row-sum
                                # matmul pass only streams nk/4 tiles
                                eng = nc.vector if (k // 4) % 2 == 0 else nc.gpsimd
                                p0 = ptm[k - 3][0]
                                p1_ = ptm[k - 2][0]
                                p2_ = ptm[k - 1][0]
                                p3 = ptm[k][0]
                                qa = ptp.tile([P, SC], MDT, tag="qa", bufs=4)
                                qb = ptp.tile([P, SC], MDT, tag="qb", bufs=4)
                                eng.tensor_add(qa[:], p0[:], p1_[:])
                                eng.tensor_add(qb[:], p2_[:], p3[:])
                                eng.tensor_add(qa[:], qa[:], qb[:])
                                red.append(qa)
                        pts = [ptm[k][0] for k in range(nk)]
                        offs = [ptm[k][1] for k in range(nk)]
                        # fill the exp-dependency window with the previous
                        # chunk's out-projection (pure PE work, no deps)
                        if prev_outT is not None:
                            emit_outproj_tgroup(prev_outT, c - 1, h)
                        # AV: outT_h [HD, sq] = sum_k V_k^T @ probsT_k
                        # (accumulated in ks order = probs completion order)
                        av_ps = ps2.tile([P, SC], FP32, tag="av")
                        for i, k in enumerate(ks):
                            nc.tensor.matmul(
                                av_ps[:, offs[k]:], v_tiles[k][:],
                                pts[k][:, offs[k]:],
                                start=(i == 0), stop=(i == nk - 1),
                            )
                        # all-ones stationary -> every psum partition gets
                        # the column sum over sk (broadcast for free).
                        # Emitted after AV so the DVE pre-reduction has the
                        # whole AV pass of cover before the PE needs it.
                        sums_ps = pss.tile([P, SC], FP32, tag="sums", bufs=1)
                        for i, rt in enumerate(red):
                            nc.tensor.matmul(
                                sums_ps[:], ones_sb[:], rt[:],
                                start=(i == 0), stop=(i == len(red) - 1),
                            )
                        # normalize: outT[h] = av * (1/sums)
                        recip = nrm.tile([P, SC], FP32, tag="recip")
                        nc.vector.reciprocal(recip[:], sums_ps[:])
                        nc.vector.tensor_mul(outT[h][:], av_ps[:], recip[:])
                    prev_outT = outT

                # drain: out-projection for the last chunk
                for t in range(SC // P):
                    emit_outproj_tgroup(prev_outT, NCH - 1, t)

    nc.compile()
    return nc


def _host_tables():
    inv_freq = 1.0 / (THETA ** (np.arange(0, HD, 2, dtype=np.float32) / HD))
    t = np.arange(S, dtype=np.float32)
    freqs = t[:, None] * inv_freq[None, :]              # [S, HD/2]
    emb = np.concatenate([freqs, freqs], axis=-1)       # [S, HD]
    cos = np.cos(emb).astype(np.float32)
    sin = np.sin(emb).astype(np.float32)
    cosT = np.ascontiguousarray(cos.T)                  # [HD, S]
    sinT = np.ascontiguousarray(sin.T)
    sinrotT = sinT.copy()
    sinrotT[0:HD // 2] = -sinT[0:HD // 2]
    return cosT, sinrotT


def get_program(mm_mode=MM_MODE, repeat=1):
    key = ("nc", mm_mode, repeat)
    if key not in _CACHE:
        _CACHE[key] = _build_program(mm_mode, repeat)
    return _CACHE[key]


def _mdt_np(mm_mode):
    return ml_dtypes.bfloat16 if mm_mode == "bf16" else np.float32


def make_in_maps(x, wq, wk, wv, wo, mask, mm_mode=MM_MODE):
    mdt = _mdt_np(mm_mode)
    x = np.asarray(x, dtype=np.float32)
    wq = np.asarray(wq, dtype=np.float32).astype(mdt)
    wk = np.asarray(wk, dtype=np.float32).astype(mdt)
    wv = np.asarray(wv, dtype=np.float32).astype(mdt)
    wo = np.asarray(wo, dtype=np.float32).astype(mdt)
    mask = np.asarray(mask, dtype=np.float32)

    cosT, sinrotT = _host_tables()
    ident = np.eye(P, dtype=np.float32)
    # maskdiag[r*128+a, b] = mask[0,0, b, r*128+a]; pattern repeats per chunk
    maskdiag = np.ascontiguousarray(mask[0, 0, 0:SC, 0:SC].T)

    xT = [np.ascontiguousarray(x[b].T).astype(mdt) for b in range(B)]
    in_maps = []
    for c in range(NCORES):
        b, g = c // NG, c % NG
        qc0 = g * NH * HD
        kc0 = g * HD
        in_maps.append({
            "xT": xT[b],
            "wqg": np.ascontiguousarray(wq[:, qc0:qc0 + NH * HD]),
            "wkg": np.ascontiguousarray(wk[:, kc0:kc0 + HD]),
            "wvg": np.ascontiguousarray(wv[:, kc0:kc0 + HD]),
            "wog": np.ascontiguousarray(wo[qc0:qc0 + NH * HD, :]),
            "cosT": cosT,
            "sinrotT": sinrotT,
            "maskdiag": maskdiag,
            "ident": ident,
            "ones": np.ones((P, P), dtype=np.float32).astype(mdt),
        })
    return in_maps


LAST_RESULTS = None


def _make_exec(nc):
    """Mirror run_bass_via_pjrt's multi-core path, but keep the jitted
    executable so repeated (timed) dispatches skip retrace/reload."""
    import jax
    from jax.experimental.shard_map import shard_map
    from jax.sharding import Mesh, PartitionSpec

    from concourse import bass2jax, mybir as _mybir

    bass2jax.install_neuronx_cc_hook()
    partition_name = (
        nc.partition_id_tensor.name if nc.partition_id_tensor else None)
    in_names, out_names, out_avals, zero_outs = [], [], [], []
    for alloc in nc.m.functions[0].allocations:
        if not isinstance(alloc, _mybir.MemoryLocationSet):
            continue
        name = alloc.memorylocations[0].name
        if alloc.kind == "ExternalInput":
            if name != partition_name:
                in_names.append(name)
        elif alloc.kind == "ExternalOutput":
            shape = tuple(alloc.tensor_shape)
            dtype = _mybir.dt.np(alloc.dtype)
            out_names.append(name)
            out_avals.append(jax.core.ShapedArray(shape, dtype))
            zero_outs.append(np.zeros(shape, dtype))
    n_params = len(in_names)
    n_outs = len(out_avals)
    all_in_names = list(in_names) + list(out_names)
    if partition_name is not None:
        all_in_names.append(partition_name)
    donate = tuple(range(n_params, n_params + n_outs))

    def _body(*args):
        operands = list(args)
        if partition_name is not None:
            operands.append(bass2jax.partition_id_tensor())
        outs = bass2jax._bass_exec_p.bind(
            *operands,
            out_avals=tuple(out_avals),
            in_names=tuple(all_in_names),
            out_names=tuple(out_names),
            lowering_input_output_aliases=(),
            sim_require_finite=True,
            sim_require_nnan=True,
            nc=nc,
        )
        return tuple(outs)

    devices = jax.devices()[:NCORES]
    mesh = Mesh(np.asarray(devices), ("core",))
    sharded = jax.jit(
        shard_map(
            _body, mesh=mesh,
            in_specs=(PartitionSpec("core"),) * (n_params + n_outs),
            out_specs=(PartitionSpec("core"),) * n_outs,
            check_rep=False,
        ),
        donate_argnums=donate, keep_unused=True,
    )
    return {
        "fn": sharded, "in_names": in_names, "out_names": out_names,
        "out_avals": out_avals, "zero_outs": zero_outs, "mesh": mesh,
    }


def get_exec(mm_mode=MM_MODE, repeat=1):
    key = ("exec", mm_mode, repeat)
    if key not in _CACHE:
        _CACHE[key] = _make_exec(get_program(mm_mode, repeat))
    return _CACHE[key]


def _concat_inputs(ex, in_maps):
    return [
        np.concatenate([np.asarray(in_maps[c][name]) for c in range(NCORES)],
                       axis=0)
        for name in ex["in_names"]
    ]


def _concat_zeros(ex):
    return [
        np.zeros((NCORES * z.shape[0], *z.shape[1:]), z.dtype)
        for z in ex["zero_outs"]
    ]


def run_on_device(in_maps, mm_mode=MM_MODE, repeat=1):
    """One dispatch; returns per-core output dicts (numpy)."""
    ex = get_exec(mm_mode, repeat)
    out_arrs = ex["fn"](*_concat_inputs(ex, in_maps), *_concat_zeros(ex))
    res = []
    for c in range(NCORES):
        res.append({
            name: np.asarray(out_arrs[i]).reshape(
                NCORES, *ex["out_avals"][i].shape)[c]
            for i, name in enumerate(ex["out_names"])
        })
    return res


def bench(in_maps, iters=5, mm_mode=MM_MODE, repeat=1):
    """Timed repeated dispatch: inputs pre-placed on device, fresh donated
    zero output buffers pre-placed per iteration. Returns list of wall ns."""
    import time

    import jax
    from jax.sharding import NamedSharding, PartitionSpec

    ex = get_exec(mm_mode, repeat)
    sh = NamedSharding(ex["mesh"], PartitionSpec("core"))
    dev_in = [jax.device_put(a, sh) for a in _concat_inputs(ex, in_maps)]
    zsets = [[jax.device_put(z, sh) for z in _concat_zeros(ex)]
             for _ in range(iters + 1)]
    jax.block_until_ready(dev_in)
    jax.block_until_ready(zsets)
    out = ex["fn"](*dev_in, *zsets[0])       # warm-up
    jax.block_until_ready(out)
    times = []
    for i in range(iters):
        t0 = time.perf_counter()
        out = ex["fn"](*dev_in, *zsets[i + 1])
        jax.block_until_ready(out)
        times.append((time.perf_counter() - t0) * 1e9)
    return times


def bench_slope(in_maps, iters=8, mm_mode=MM_MODE, r_hi=4):
    """Per-iteration kernel time via slope: (T(r_hi) - T(1)) / (r_hi - 1),
    immune to constant dispatch overhead.

    Two noise sources dominate the axon dispatch wall: slow drift of the
    ~70-90 ms overhead, and an executable-switch cost paid by the first
    dispatch after changing NEFFs (size-dependent, so it biases the slope).
    So: run same-executable BATCHES, alternate batches between the two
    executables (cancels drift at batch granularity), drop the first
    dispatch of every batch (absorbs the switch cost), and take the slope
    of the medians of the surviving samples.
    """
    import time

    import jax
    from jax.sharding import NamedSharding, PartitionSpec

    def prep(ex):
        sh = NamedSharding(ex["mesh"], PartitionSpec("core"))
        dev_in = [jax.device_put(a, sh) for a in _concat_inputs(ex, in_maps)]
        zsets = [[jax.device_put(z, sh) for z in _concat_zeros(ex)]
                 for _ in range(iters + 4)]
        jax.block_until_ready(dev_in)
        jax.block_until_ready(zsets)
        return [ex, dev_in, zsets, 0, []]

    s1 = prep(get_exec(mm_mode, 1))
    sh_ = prep(get_exec(mm_mode, r_hi))
    # warm-up both executables once
    for s in (s1, sh_):
        out = s[0]["fn"](*s[1], *s[2][s[3]])
        jax.block_until_ready(out)
        s[3] += 1

    nbatch = 3
    bs = max(2, iters // nbatch)
    for b in range(nbatch):
        for s in (s1, sh_):
            ex, dev_in, zsets, zi, store = s
            for j in range(bs + 1):
                if zi >= len(zsets):
                    break
                t0 = time.perf_counter()
                out = ex["fn"](*dev_in, *zsets[zi])
                jax.block_until_ready(out)
                dt = (time.perf_counter() - t0) * 1e9
                s[3] = zi = zi + 1
                if j > 0:      # first dispatch pays the NEFF switch
                    store.append(dt)

    t1s, ths = s1[4], sh_[4]
    slope = (np.median(ths) - np.median(t1s)) / (r_hi - 1)
    return {
        "t1": t1s, "th": ths,
        "exec_ns_median": float(slope),
        "exec_ns_min": float(slope),
    }


def kernel(x, wq, wk, wv, wo, mask):
    """Full inputs in, full output out; shards over the 8 NeuronCores."""
    global LAST_RESULTS
    from concourse import bass_utils

    nc = get_program()
    in_maps = make_in_maps(x, wq, wk, wv, wo, mask)
    res = bass_utils.run_bass_kernel_spmd(
        nc, in_maps, core_ids=list(range(NCORES)))
    LAST_RESULTS = res
    out = np.zeros((B, S, D), dtype=np.float32)
    for c in range(NCORES):
        b = c // NG
        out[b] += np.asarray(res.results[c]["y"]).astype(np.float32)
    return out


# revision 23
# speedup vs baseline: 1.0032x; 1.0032x over previous
"""Trainium2 Bass kernel for GQA attention (B=2,S=2048,D=2048,H=16,KV=4,HD=128)
with RoPE + causal mask, sharded over 8 NeuronCores:
  2-way data parallel over batch x 4-way tensor parallel over KV groups.

Core c = (b, g): b = c // 4, g = c % 4.
Each core computes, for its batch b and KV group g (q heads 4g..4g+3):
  QT_h [HD,S], KT [HD,S] (RoPE'd), V [S,HD]    via matmul vs xT [D,S]
  scoresT [sk,sq] blocks, exp on ScalarE (scale folded), row-sums via an
  all-ones matmul (which also replicates the sums across partitions),
  AV with V tiles stationary -> outT [HD,sq], per-head normalization via
  reciprocal, partial y = attn_norm @ wo_rows[g]; host sums the 4 partials.

matmul(out, lhsT, rhs) = lhsT.T @ rhs, contraction over the partition dim.
All contractions are K=128.  Causality at block granularity: fully-masked
(sk,sq) blocks skipped; diagonal blocks add the mask slice (pattern repeats
every 4 sk-tiles, so only a [512,512] mask transpose is shipped).

Matmul operands are bf16 (full-kernel relative error ~3e-3 vs the fp32
reference; the gate is 2e-2).  All accumulation is fp32 in PSUM; softmax
exp/normalization arithmetic is fp32.  The output projection for sq-chunk c
is software-pipelined one chunk behind attention so its PE work fills the
softmax-dependency bubbles of chunk c+1, and y tiles are DMA'd straight
from PSUM.
"""

import os
from contextlib import ExitStack

import numpy as np
import ml_dtypes

import concourse.bacc as bacc
import concourse.mybir as mybir
import concourse.tile as tile

# ---------------- problem constants (hardcoded per contract) ----------------
B, S, D = 2, 2048, 2048
H, KV, HD = 16, 4, 128
REP = H // KV            # 4 q heads per kv head
NG = KV                  # 4 tensor-parallel groups
NCORES = 8
THETA = 10000.0
SCALE = 1.0 / float(np.sqrt(HD))

P = 128                  # partition dim
SC = 512                 # moving free-dim chunk (one PSUM bank in fp32)
NDT = S // P             # 16 tiles of 128 along S or D
NCH = S // SC            # 4 chunks of 512 along S
NH = REP                 # 4 q-heads per core

FP32 = mybir.dt.float32
BF16 = mybir.dt.bfloat16

# matmul dtype: "bf16" (measured full-kernel relative error ~3e-3 vs the
# fp32 reference -- gate is 2e-2) or "fp32" (exact to ~1e-6, 4 cyc/row).
MM_MODE = os.environ.get("KERNEL_MM_MODE", "bf16")

_CACHE = {}


def _build_program(mm_mode=MM_MODE, repeat=1):
    MDT = BF16 if mm_mode == "bf16" else FP32

    nc = bacc.Bacc("TRN2", target_bir_lowering=False, debug=False)

    xT_d = nc.dram_tensor("xT", [D, S], MDT, kind="ExternalInput").ap()
    wq_d = nc.dram_tensor("wqg", [D, NH * HD], MDT, kind="ExternalInput").ap()
    wk_d = nc.dram_tensor("wkg", [D, HD], MDT, kind="ExternalInput").ap()
    wv_d = nc.dram_tensor("wvg", [D, HD], MDT, kind="ExternalInput").ap()
    wo_d = nc.dram_tensor("wog", [NH * HD, D], MDT, kind="ExternalInput").ap()
    cosT_d = nc.dram_tensor("cosT", [HD, S], FP32, kind="ExternalInput").ap()
    sinrT_d = nc.dram_tensor("sinrotT", [HD, S], FP32, kind="ExternalInput").ap()
    mdiag_d = nc.dram_tensor("maskdiag", [NCH * P, SC], FP32, kind="ExternalInput").ap()
    ident_d = nc.dram_tensor("ident", [P, P], FP32, kind="ExternalInput").ap()
    ones_d = nc.dram_tensor("ones", [P, P], MDT, kind="ExternalInput").ap()
    y_d = nc.dram_tensor("y", [S, D], MDT, kind="ExternalOutput").ap()

    with tile.TileContext(nc) as tc, ExitStack() as ctx:
        qkv = ctx.enter_context(tc.tile_pool(name="qkv", bufs=1))
        small = ctx.enter_context(tc.tile_pool(name="small", bufs=1))

        # resident Q^T per head, K^T, V tiles
        qt = [qkv.tile([P, S], MDT, tag=f"qt{h}", name=f"qt{h}") for h in range(NH)]
        kt = qkv.tile([P, S], MDT, tag="kt", name="kt")
        v_tiles = [qkv.tile([P, HD], MDT, tag=f"v{k}", name=f"v{k}")
                   for k in range(NDT)]

        ones_sb = small.tile([P, P], MDT, tag="ones")
        ident_sb = small.tile([P, P], FP32, tag="ident")
        mdiag_slab = small.tile([P, NCH * SC], FP32, tag="mds")
        mdiag_sb = [mdiag_slab[:, r * SC:(r + 1) * SC] for r in range(NCH)]
        wos = small.tile([P, NH * D], MDT, tag="wos")
        wo_sb = [wos[:, h * D:(h + 1) * D] for h in range(NH)]

        def load_consts():
            nc.gpsimd.dma_start(ones_sb[:], ones_d[:])
            nc.gpsimd.dma_start(ident_sb[:], ident_d[:])
            nc.gpsimd.dma_start(
                mdiag_slab[:].rearrange("p (r s) -> p r s", r=NCH),
                mdiag_d.rearrange("(r p) s -> p r s", p=P))

        for rep in range(repeat):
            # ============== phase 1: QKV projection + RoPE ==============
            with tc.tile_pool(name="p1", bufs=1) as p1, \
                 tc.tile_pool(name="xin", bufs=8) as xin, \
                 tc.tile_pool(name="rtmp", bufs=3) as rtmp, \
                 tc.tile_pool(name="ps1", bufs=2, space="PSUM") as ps1:

                # weight slab, head-major: head m's d-tile k lives at slab
                # columns [m*NDT*HD + k*HD, ...).  One DMA per head, head 0
                # first, so the first matmul chain is gated by x-chunk0 +
                # head-0 weights only (~2.5 MB in bf16).
                XQ = NDT // 4   # 4 d-tiles per quarter x slab
                wqs = p1.tile([P, NH * NDT * HD], MDT, tag="wqs")
                wks = p1.tile([P, NDT * HD], MDT, tag="wks")
                wvs = p1.tile([P, NDT * HD], MDT, tag="wvs")
                cosT_sb = p1.tile([HD, S], FP32, tag="cosT")
                sinrT_sb = p1.tile([HD, S], FP32, tag="sinrT")
                vT = p1.tile([HD, S], FP32, tag="vT")

                # All phase-1 loads ride the sync queue: it is idle during
                # phase 2, so the NEXT rep's weight prefetch drains while
                # attention runs (scalar/gpsimd queues are FIFO-blocked
                # behind phase-2 work, which would stall the rep boundary).
                def load_wq_head(m):
                    nc.sync.dma_start(
                        wqs[:, m * NDT * HD:(m + 1) * NDT * HD]
                        .rearrange("p (n q) -> p n q", n=NDT),
                        wq_d[:, m * HD:(m + 1) * HD]
                        .rearrange("(n p) q -> p n q", p=P))

                if rep == 0:
                    # warm the PE p-state with dummy matmuls on a memset
                    # scratch tile while the first input DMAs are in flight
                    wsrc = p1.tile([P, P], MDT, tag="wsrc")
                    nc.gpsimd.memset(wsrc[:], 0.0)
                    for i in range(48):
                        wps = ps1.tile([P, SC], FP32, tag="proj", bufs=4)
                        nc.tensor.matmul(wps[:, 0:P], wsrc[:], wsrc[:],
                                         start=True, stop=True)
                load_wq_head(0)

                for sc in range(NCH):
                    s0 = sc * SC
                    xq_slabs = []
                    for qq in range(4):
                        xs = xin.tile([P, XQ * SC], MDT, tag="x")
                        nc.sync.dma_start(
                            xs[:].rearrange("p (n s) -> p n s", n=XQ),
                            xT_d[qq * XQ * P:(qq + 1) * XQ * P, s0:s0 + SC]
                            .rearrange("(n p) s -> p n s", p=P))
                        xq_slabs.append(xs)
                    if sc == 0:
                        # phase-1/2 weights + tables: issue after chunk-0 x
                        # so the first chain starts asap; RoPE tables first
                        # (needed by m=0's RoPE), then the later weights
                        nc.sync.dma_start(cosT_sb[:], cosT_d[:])
                        nc.sync.dma_start(sinrT_sb[:], sinrT_d[:])
                        for m in range(1, NH):
                            load_wq_head(m)
                        nc.sync.dma_start(
                            wks[:].rearrange("p (n m) -> p n m", n=NDT),
                            wk_d.rearrange("(n p) m -> p n m", p=P))
                        nc.sync.dma_start(
                            wvs[:].rearrange("p (n m) -> p n m", n=NDT),
                            wv_d.rearrange("(n p) m -> p n m", p=P))
                        if rep == 0:
                            load_consts()
                        nc.sync.dma_start(
                            wos[:].rearrange("p (n d) -> p n d", n=NH),
                            wo_d.rearrange("(n p) d -> p n d", p=P))

                    def xts_k(k):
                        return xq_slabs[k // XQ][:, (k % XQ) * SC:(k % XQ + 1) * SC]

                    # m = 0..3: q heads; 4: k; 5: v
                    for m in range(NH + 2):
                        psum = ps1.tile([P, SC], FP32, tag="proj", bufs=4)
                        for k in range(NDT):
                            if m < NH:
                                lhsT = wqs[:, m * NDT * HD + k * HD:
                                           m * NDT * HD + (k + 1) * HD]
                            elif m == NH:
                                lhsT = wks[:, k * HD:(k + 1) * HD]
                            else:
                                lhsT = wvs[:, k * HD:(k + 1) * HD]
                            nc.tensor.matmul(
                                psum[:], lhsT, xts_k(k),
                                start=(k == 0), stop=(k == NDT - 1),
                            )
                        if m <= NH:
                            # RoPE: dst = psum*cosT + shift(psum)*sinrotT
                            dst = (qt[m] if m < NH else kt)[:, s0:s0 + SC]
                            t0 = rtmp.tile([P, SC], FP32, tag="t0")
                            t1 = rtmp.tile([P, SC], FP32, tag="t1")
                            nc.vector.tensor_mul(
                                t0[:], psum[:], cosT_sb[:, s0:s0 + SC])
                            nc.vector.tensor_mul(
                                t1[0:64, :], psum[64:128, :],
                                sinrT_sb[0:64, s0:s0 + SC])
                            nc.vector.tensor_mul(
                                t1[64:128, :], psum[0:64, :],
                                sinrT_sb[64:128, s0:s0 + SC])
                            nc.vector.tensor_add(dst, t0[:], t1[:])
                        else:
                            nc.vector.tensor_copy(vT[:, s0:s0 + SC], psum[:])

                    # transpose this chunk of V^T -> V tiles [S_k=128, HD]
                    for kk in range(SC // P):
                        k = sc * (SC // P) + kk
                        ps_t = ps1.tile([P, P], FP32, tag="vt")
                        nc.tensor.transpose(
                            ps_t[:], vT[:, k * P:(k + 1) * P], ident_sb[:])
                        nc.vector.tensor_copy(v_tiles[k][:], ps_t[:])

            # ========== phase 2: attention + output projection ==========
            # The out-projection for chunk c-1 is emitted interleaved into
            # chunk c's attention (one t-group of 16 matmuls after each
            # head) so the PE never waits on the softmax/normalize chains.
            with tc.tile_pool(name="p2", bufs=2) as p2, \
                 tc.tile_pool(name="pt", bufs=24) as ptp, \
                 tc.tile_pool(name="nrm", bufs=4) as nrm, \
                 tc.tile_pool(name="yst", bufs=4) as yst, \
                 tc.tile_pool(name="ps2", bufs=2, space="PSUM") as ps2, \
                 tc.tile_pool(name="pss", bufs=2, space="PSUM") as pss:

                prev_outT = None

                def emit_outproj_tgroup(outT_tiles, c_prev, t):
                    q0p = c_prev * SC
                    for dci in range(NCH):
                        d0 = dci * SC
                        y_ps = ps2.tile([P, SC], FP32, tag="y", bufs=2)
                        for h in range(NH):
                            nc.tensor.matmul(
                                y_ps[:],
                                outT_tiles[h][:, t * P:(t + 1) * P],
                                wo_sb[h][:, d0:d0 + SC],
                                start=(h == 0), stop=(h == NH - 1),
                            )
                        y_sb = yst.tile([P, SC], MDT, tag="ysb")
                        # alternate the PSUM->SBUF copy between the two
                        # element-wise engines; Act is exp-saturated
                        if dci % 2 == 0:
                            nc.vector.tensor_copy(y_sb[:], y_ps[:])
                        else:
                            nc.scalar.activation(
                                y_sb[:], y_ps[:],
                                mybir.ActivationFunctionType.Copy)
                        row0 = q0p + t * P
                        nc.gpsimd.dma_start(
                            y_d[row0:row0 + P, d0:d0 + SC], y_sb[:])

                for c in range(NCH):
                    q0 = c * SC
                    nk = 4 * c + 4          # active sk tiles (causal)
                    outT = [p2.tile([P, SC], MDT, tag=f"ot{h}",
                                    name=f"ot{h}") for h in range(NH)]
                    for h in range(NH):
                        ptm = {}             # k -> (pt tile, off)
                        red = []             # pre-reduced tiles for row sums
                        ks = list(range(nk))
                        for k in ks:
                            # diagonal blocks: sk tile k only attends to
                            # sq >= 128k, i.e. chunk columns [off:512); only
                            # the leading 128 columns of that are a partial
                            # (triangular) mask -- the rest is fully allowed.
                            off = max(0, (k - 4 * c) * P)
                            sc_ps = ps2.tile([P, SC], FP32, tag="sc", bufs=3)
                            nc.tensor.matmul(
                                sc_ps[:, off:],
                                kt[:, k * P:(k + 1) * P],
                                qt[h][:, q0 + off:q0 + SC],
                                start=True, stop=True,
                            )
                            pt = ptp.tile([P, SC], MDT, tag="pt")
                            if k >= 4 * c:
                                # triangle columns [off:off+128): scale+mask
                                # on DVE then exp; columns beyond are plain
                                r = k % NCH
                                nc.vector.scalar_tensor_tensor(
                                    sc_ps[:, off:off + P], sc_ps[:, off:off + P],
                                    SCALE, mdiag_sb[r][:, off:off + P],
                                    op0=mybir.AluOpType.mult,
                                    op1=mybir.AluOpType.add)
                                nc.scalar.activation(
                                    pt[:, off:off + P], sc_ps[:, off:off + P],
                                    mybir.ActivationFunctionType.Exp)
                                if off + P < SC:
                                    nc.scalar.activation(
                                        pt[:, off + P:], sc_ps[:, off + P:],
                                        mybir.ActivationFunctionType.Exp,
                                        scale=SCALE)
                            else:
                                nc.scalar.activation(
                                    pt[:, off:], sc_ps[:, off:],
                                    mybir.ActivationFunctionType.Exp,
                                    scale=SCALE)
                            ptm[k] = (pt, off)
                            if k == 4 * c + 3:
                                # staircase-sum the 4 diagonal prob tiles
                                d0 = ptm[4 * c][0]
                                d1 = ptm[4 * c + 1][0]
                                d2 = ptm[4 * c + 2][0]
                                d3 = ptm[4 * c + 3][0]
                                ds = ptp.tile([P, SC], MDT, tag="ds", bufs=4)
                                nc.vector.tensor_copy(ds[:, 0:P], d0[:, 0:P])
                                nc.vector.tensor_add(
                                    ds[:, P:], d0[:, P:], d1[:, P:])
                                nc.vector.tensor_add(
                                    ds[:, 2 * P:], ds[:, 2 * P:], d2[:, 2 * P:])
                                nc.vector.tensor_add(
                                    ds[:, 3 * P:], ds[:, 3 * P:], d3[:, 3 * P:])
                                red.append(ds)
                            if k % 4 == 3 and k < 4 * c:
                                # quad-reduce 4 full off-diagonal prob tiles
                                # (alternating DVE / gpsimd) so the row-sum
                                # matmul pass only streams nk/4 tiles
                                eng = nc.vector if (k // 4) % 2 == 0 else nc.gpsimd
                                p0 = ptm[k - 3][0]
                                p1_ = ptm[k - 2][0]
                                p2_ = ptm[k - 1][0]
                                p3 = ptm[k][0]
                                qa = ptp.tile([P, SC], MDT, tag="qa", bufs=4)
                                qb = ptp.tile([P, SC], MDT, tag="qb", bufs=4)
                                eng.tensor_add(qa[:], p0[:], p1_[:])
                                eng.tensor_add(qb[:], p2_[:], p3[:])
                                eng.tensor_add(qa[:], qa[:], qb[:])
                                red.append(qa)
                        pts = [ptm[k][0] for k in range(nk)]
                        offs = [ptm[k][1] for k in range(nk)]
                        # fill the exp-dependency window with the previous
                        # chunk's out-projection (pure PE work, no deps)
                        if prev_outT is not None:
                            emit_outproj_tgroup(prev_outT, c - 1, h)
                        # AV: outT_h [HD, sq] = sum_k V_k^T @ probsT_k
                        # (accumulated in ks order = probs completion order)
                        av_ps = ps2.tile([P, SC], FP32, tag="av")
                        for i, k in enumerate(ks):
                            nc.tensor.matmul(
                                av_ps[:, offs[k]:], v_tiles[k][:],
                                pts[k][:, offs[k]:],
                                start=(i == 0), stop=(i == nk - 1),
                            )
                        # all-ones stationary -> every psum partition gets
                        # the column sum over sk (broadcast for free).
                        # Emitted after AV so the DVE pre-reduction has the
                        # whole AV pass of cover before the PE needs it.
                        sums_ps = pss.tile([P, SC], FP32, tag="sums", bufs=1)
                        for i, rt in enumerate(red):
                            nc.tensor.matmul(
                                sums_ps[:], ones_sb[:], rt[:],
                                start=(i == 0), stop=(i == len(red) - 1),
                            )
                        # normalize: outT[h] = av * (1/sums)
                        recip = nrm.tile([P, SC], FP32, tag="recip")
                        nc.vector.reciprocal(recip[:], sums_ps[:])
                        nc.vector.tensor_mul(outT[h][:], av_ps[:], recip[:])
                    prev_outT = outT

                # drain: out-projection for the last chunk
                for t in range(SC // P):
                    emit_outproj_tgroup(prev_outT, NCH - 1, t)

    nc.compile()
    return nc


def _host_tables():
    inv_freq = 1.0 / (THETA ** (np.arange(0, HD, 2, dtype=np.float32) / HD))
    t = np.arange(S, dtype=np.float32)
    freqs = t[:, None] * inv_freq[None, :]              # [S, HD/2]
    emb = np.concatenate([freqs, freqs], axis=-1)       # [S, HD]
    cos = np.cos(emb).astype(np.float32)
    sin = np.sin(emb).astype(np.float32)
    cosT = np.ascontiguousarray(cos.T)                  # [HD, S]
    sinT = np.ascontiguousarray(sin.T)
    sinrotT = sinT.copy()
    sinrotT[0:HD // 2] = -sinT[0:HD // 2]
    return cosT, sinrotT


def get_program(mm_mode=MM_MODE, repeat=1):
    key = ("nc", mm_mode, repeat)
    if key not in _CACHE:
        _CACHE[key] = _build_program(mm_mode, repeat)
    return _CACHE[key]


def _mdt_np(mm_mode):
    return ml_dtypes.bfloat16 if mm_mode == "bf16" else np.float32


def make_in_maps(x, wq, wk, wv, wo, mask, mm_mode=MM_MODE):
    mdt = _mdt_np(mm_mode)
    x = np.asarray(x, dtype=np.float32)
    wq = np.asarray(wq, dtype=np.float32).astype(mdt)
    wk = np.asarray(wk, dtype=np.float32).astype(mdt)
    wv = np.asarray(wv, dtype=np.float32).astype(mdt)
    wo = np.asarray(wo, dtype=np.float32).astype(mdt)
    mask = np.asarray(mask, dtype=np.float32)

    cosT, sinrotT = _host_tables()
    ident = np.eye(P, dtype=np.float32)
    # maskdiag[r*128+a, b] = mask[0,0, b, r*128+a]; pattern repeats per chunk
    maskdiag = np.ascontiguousarray(mask[0, 0, 0:SC, 0:SC].T)

    xT = [np.ascontiguousarray(x[b].T).astype(mdt) for b in range(B)]
    in_maps = []
    for c in range(NCORES):
        b, g = c // NG, c % NG
        qc0 = g * NH * HD
        kc0 = g * HD
        in_maps.append({
            "xT": xT[b],
            "wqg": np.ascontiguousarray(wq[:, qc0:qc0 + NH * HD]),
            "wkg": np.ascontiguousarray(wk[:, kc0:kc0 + HD]),
            "wvg": np.ascontiguousarray(wv[:, kc0:kc0 + HD]),
            "wog": np.ascontiguousarray(wo[qc0:qc0 + NH * HD, :]),
            "cosT": cosT,
            "sinrotT": sinrotT,
            "maskdiag": maskdiag,
            "ident": ident,
            "ones": np.ones((P, P), dtype=np.float32).astype(mdt),
        })
    return in_maps


LAST_RESULTS = None


def _make_exec(nc):
    """Mirror run_bass_via_pjrt's multi-core path, but keep the jitted
    executable so repeated (timed) dispatches skip retrace/reload."""
    import jax
    from jax.experimental.shard_map import shard_map
    from jax.sharding import Mesh, PartitionSpec

    from concourse import bass2jax, mybir as _mybir

    bass2jax.install_neuronx_cc_hook()
    partition_name = (
        nc.partition_id_tensor.name if nc.partition_id_tensor else None)
    in_names, out_names, out_avals, zero_outs = [], [], [], []
    for alloc in nc.m.functions[0].allocations:
        if not isinstance(alloc, _mybir.MemoryLocationSet):
            continue
        name = alloc.memorylocations[0].name
        if alloc.kind == "ExternalInput":
            if name != partition_name:
                in_names.append(name)
        elif alloc.kind == "ExternalOutput":
            shape = tuple(alloc.tensor_shape)
            dtype = _mybir.dt.np(alloc.dtype)
            out_names.append(name)
            out_avals.append(jax.core.ShapedArray(shape, dtype))
            zero_outs.append(np.zeros(shape, dtype))
    n_params = len(in_names)
    n_outs = len(out_avals)
    all_in_names = list(in_names) + list(out_names)
    if partition_name is not None:
        all_in_names.append(partition_name)
    donate = tuple(range(n_params, n_params + n_outs))

    def _body(*args):
        operands = list(args)
        if partition_name is not None:
            operands.append(bass2jax.partition_id_tensor())
        outs = bass2jax._bass_exec_p.bind(
            *operands,
            out_avals=tuple(out_avals),
            in_names=tuple(all_in_names),
            out_names=tuple(out_names),
            lowering_input_output_aliases=(),
            sim_require_finite=True,
            sim_require_nnan=True,
            nc=nc,
        )
        return tuple(outs)

    devices = jax.devices()[:NCORES]
    mesh = Mesh(np.asarray(devices), ("core",))
    sharded = jax.jit(
        shard_map(
            _body, mesh=mesh,
            in_specs=(PartitionSpec("core"),) * (n_params + n_outs),
            out_specs=(PartitionSpec("core"),) * n_outs,
            check_rep=False,
        ),
        donate_argnums=donate, keep_unused=True,
    )
    return {
        "fn": sharded, "in_names": in_names, "out_names": out_names,
        "out_avals": out_avals, "zero_outs": zero_outs, "mesh": mesh,
    }


def get_exec(mm_mode=MM_MODE, repeat=1):
    key = ("exec", mm_mode, repeat)
    if key not in _CACHE:
        _CACHE[key] = _make_exec(get_program(mm_mode, repeat))
    return _CACHE[key]


def _concat_inputs(ex, in_maps):
    return [
        np.concatenate([np.asarray(in_maps[c][name]) for c in range(NCORES)],
                       axis=0)
        for name in ex["in_names"]
    ]


def _concat_zeros(ex):
    return [
        np.zeros((NCORES * z.shape[0], *z.shape[1:]), z.dtype)
        for z in ex["zero_outs"]
    ]


def run_on_device(in_maps, mm_mode=MM_MODE, repeat=1):
    """One dispatch; returns per-core output dicts (numpy)."""
    ex = get_exec(mm_mode, repeat)
    out_arrs = ex["fn"](*_concat_inputs(ex, in_maps), *_concat_zeros(ex))
    res = []
    for c in range(NCORES):
        res.append({
            name: np.asarray(out_arrs[i]).reshape(
                NCORES, *ex["out_avals"][i].shape)[c]
            for i, name in enumerate(ex["out_names"])
        })
    return res


def bench(in_maps, iters=5, mm_mode=MM_MODE, repeat=1):
    """Timed repeated dispatch: inputs pre-placed on device, fresh donated
    zero output buffers pre-placed per iteration. Returns list of wall ns."""
    import time

    import jax
    from jax.sharding import NamedSharding, PartitionSpec

    ex = get_exec(mm_mode, repeat)
    sh = NamedSharding(ex["mesh"], PartitionSpec("core"))
    dev_in = [jax.device_put(a, sh) for a in _concat_inputs(ex, in_maps)]
    zsets = [[jax.device_put(z, sh) for z in _concat_zeros(ex)]
             for _ in range(iters + 1)]
    jax.block_until_ready(dev_in)
    jax.block_until_ready(zsets)
    out = ex["fn"](*dev_in, *zsets[0])       # warm-up
    jax.block_until_ready(out)
    times = []
    for i in range(iters):
        t0 = time.perf_counter()
        out = ex["fn"](*dev_in, *zsets[i + 1])
        jax.block_until_ready(out)
        times.append((time.perf_counter() - t0) * 1e9)
    return times


def bench_slope(in_maps, iters=8, mm_mode=MM_MODE, r_hi=4):
    """Per-iteration kernel time via slope: (T(r_hi) - T(1)) / (r_hi - 1),
    immune to constant dispatch overhead.

    Two noise sources dominate the axon dispatch wall: slow drift of the
    ~70-90 ms overhead, and an executable-switch cost paid by the first
    dispatch after changing NEFFs (size-dependent, so it biases the slope).
    So: run same-executable BATCHES, alternate batches between the two
    executables (cancels drift at batch granularity), drop the first
    dispatch of every batch (absorbs the switch cost), and take the slope
    of the medians of the surviving samples.
    """
    import time

    import jax
    from jax.sharding import NamedSharding, PartitionSpec

    def prep(ex):
        sh = NamedSharding(ex["mesh"], PartitionSpec("core"))
        dev_in = [jax.device_put(a, sh) for a in _concat_inputs(ex, in_maps)]
        zsets = [[jax.device_put(z, sh) for z in _concat_zeros(ex)]
                 for _ in range(iters + 4)]
        jax.block_until_ready(dev_in)
        jax.block_until_ready(zsets)
        return [ex, dev_in, zsets, 0, []]

    s1 = prep(get_exec(mm_mode, 1))
    sh_ = prep(get_exec(mm_mode, r_hi))
    # warm-up both executables once
    for s in (s1, sh_):
        out = s[0]["fn"](*s[1], *s[2][s[3]])
        jax.block_until_ready(out)
        s[3] += 1

    nbatch = 3
    bs = max(2, iters // nbatch)
    for b in range(nbatch):
        for s in (s1, sh_):
            ex, dev_in, zsets, zi, store = s
            for j in range(bs + 1):
                if zi >= len(zsets):
                    break
                t0 = time.perf_counter()
                out = ex["fn"](*dev_in, *zsets[zi])
                jax.block_until_ready(out)
                dt = (time.perf_counter() - t0) * 1e9
                s[3] = zi = zi + 1
                if j > 0:      # first dispatch pays the NEFF switch
                    store.append(dt)

    t1s, ths = s1[4], sh_[4]
    slope = (np.median(ths) - np.median(t1s)) / (r_hi - 1)
    return {
        "t1": t1s, "th": ths,
        "exec_ns_median": float(slope),
        "exec_ns_min": float(slope),
    }


def kernel(x, wq, wk, wv, wo, mask):
    """Full inputs in, full output out; shards over the 8 NeuronCores."""
    global LAST_RESULTS
    from concourse import bass_utils

    nc = get_program()
    in_maps = make_in_maps(x, wq, wk, wv, wo, mask)
    res = bass_utils.run_bass_kernel_spmd(
        nc, in_maps, core_ids=list(range(NCORES)))
    LAST_RESULTS = res
    out = np.zeros((B, S, D), dtype=np.float32)
    for c in range(NCORES):
        b = c // NG
        out[b] += np.asarray(res.results[c]["y"]).astype(np.float32)
    return out


# revision 35
# speedup vs baseline: 1.0499x; 1.0465x over previous
"""Trainium2 Bass kernel for GQA attention (B=2,S=2048,D=2048,H=16,KV=4,HD=128)
with RoPE + causal mask, sharded over 8 NeuronCores:
  2-way data parallel over batch x 4-way tensor parallel over KV groups.

Core c = (b, g): b = c // 4, g = c % 4.
Each core computes, for its batch b and KV group g (q heads 4g..4g+3):
  QT_h [HD,S], KT [HD,S] (RoPE'd), V [S,HD]    via matmul vs xT [D,S]
  scoresT [sk,sq] blocks, exp on ScalarE (scale folded), row-sums via an
  all-ones matmul (which also replicates the sums across partitions),
  AV with V tiles stationary -> outT [HD,sq], per-head normalization via
  reciprocal, partial y = attn_norm @ wo_rows[g]; host sums the 4 partials.

matmul(out, lhsT, rhs) = lhsT.T @ rhs, contraction over the partition dim.
All contractions are K=128.  Causality at block granularity: fully-masked
(sk,sq) blocks skipped; diagonal blocks add the mask slice (pattern repeats
every 4 sk-tiles, so only a [512,512] mask transpose is shipped).

Matmul operands are bf16 (full-kernel relative error ~3e-3 vs the fp32
reference; the gate is 2e-2).  All accumulation is fp32 in PSUM; softmax
exp/normalization arithmetic is fp32.  The output projection for sq-chunk c
is software-pipelined one chunk behind attention so its PE work fills the
softmax-dependency bubbles of chunk c+1, and y tiles are DMA'd straight
from PSUM.
"""

import os
from contextlib import ExitStack

import numpy as np
import ml_dtypes

import concourse.bacc as bacc
import concourse.mybir as mybir
import concourse.tile as tile

# ---------------- problem constants (hardcoded per contract) ----------------
B, S, D = 2, 2048, 2048
H, KV, HD = 16, 4, 128
REP = H // KV            # 4 q heads per kv head
NG = KV                  # 4 tensor-parallel groups
NCORES = 8
THETA = 10000.0
SCALE = 1.0 / float(np.sqrt(HD))

P = 128                  # partition dim
SC = 512                 # moving free-dim chunk (one PSUM bank in fp32)
NDT = S // P             # 16 tiles of 128 along S or D
NCH = S // SC            # 4 chunks of 512 along S
NH = REP                 # 4 q-heads per core

FP32 = mybir.dt.float32
BF16 = mybir.dt.bfloat16

# matmul dtype: "bf16" (measured full-kernel relative error ~3e-3 vs the
# fp32 reference -- gate is 2e-2) or "fp32" (exact to ~1e-6, 4 cyc/row).
MM_MODE = os.environ.get("KERNEL_MM_MODE", "bf16")

_CACHE = {}


def _build_program(mm_mode=MM_MODE, repeat=1):
    MDT = BF16 if mm_mode == "bf16" else FP32

    nc = bacc.Bacc("TRN2", target_bir_lowering=False, debug=False)

    xT_d = nc.dram_tensor("xT", [D, S], MDT, kind="ExternalInput").ap()
    wq_d = nc.dram_tensor("wqg", [D, NH * HD], MDT, kind="ExternalInput").ap()
    wk_d = nc.dram_tensor("wkg", [D, HD], MDT, kind="ExternalInput").ap()
    wv_d = nc.dram_tensor("wvg", [D, HD], MDT, kind="ExternalInput").ap()
    wo_d = nc.dram_tensor("wog", [NH * HD, D], MDT, kind="ExternalInput").ap()
    cosT_d = nc.dram_tensor("cosT", [HD, S], FP32, kind="ExternalInput").ap()
    sinrT_d = nc.dram_tensor("sinrotT", [HD, S], FP32, kind="ExternalInput").ap()
    mdiag_d = nc.dram_tensor("maskdiag", [NCH * P, SC], FP32, kind="ExternalInput").ap()
    ident_d = nc.dram_tensor("ident", [P, P], FP32, kind="ExternalInput").ap()
    ones_d = nc.dram_tensor("ones", [P, P], MDT, kind="ExternalInput").ap()
    y_d = nc.dram_tensor("y", [S, D], MDT, kind="ExternalOutput").ap()

    with tile.TileContext(nc) as tc, ExitStack() as ctx:
        qkv = ctx.enter_context(tc.tile_pool(name="qkv", bufs=1))
        small = ctx.enter_context(tc.tile_pool(name="small", bufs=1))
        # One persistent PSUM pool for the whole program: per-rep pool
        # open/close emits engine drains that serialize rep boundaries.
        # 8 banks: mm(proj+scores) 3, av 2, vy(V-transpose+y-out) 2, sums 1.
        psp = ctx.enter_context(tc.tile_pool(name="psp", bufs=1, space="PSUM"))

        # resident Q^T per head, K^T, V tiles
        qt = [qkv.tile([P, S], MDT, tag=f"qt{h}", name=f"qt{h}") for h in range(NH)]
        kt = qkv.tile([P, S], MDT, tag="kt", name="kt")
        v_tiles = [qkv.tile([P, HD], MDT, tag=f"v{k}", name=f"v{k}")
                   for k in range(NDT)]

        ones_sb = small.tile([P, P], MDT, tag="ones")
        ident_sb = small.tile([P, P], FP32, tag="ident")
        mdiag_slab = small.tile([P, NCH * SC], FP32, tag="mds")
        mdiag_sb = [mdiag_slab[:, r * SC:(r + 1) * SC] for r in range(NCH)]
        wos = small.tile([P, NH * D], MDT, tag="wos")
        wo_sb = [wos[:, h * D:(h + 1) * D] for h in range(NH)]

        # weights + tables are rep-invariant: persistent tiles, loaded once
        # (all on the sync queue -- it idles during phase 2, so nothing
        # blocks at a rep boundary)
        wqs = small.tile([P, NH * NDT * HD], MDT, tag="wqs")
        wks = small.tile([P, NDT * HD], MDT, tag="wks")
        wvs = small.tile([P, NDT * HD], MDT, tag="wvs")
        cosT_sb = small.tile([HD, S], FP32, tag="cosT")
        sinrT_sb = small.tile([HD, S], FP32, tag="sinrT")
        # x chunk 0 is persistent too: its reload for rep r+1 only waits on
        # rep r's early phase-1 reads, never on the phase-2 pool teardown
        x0_slabs = [small.tile([P, (NDT // 4) * SC], MDT, tag=f"x0q{qq}",
                               name=f"x0q{qq}") for qq in range(4)]

        def load_consts():
            nc.gpsimd.dma_start(ones_sb[:], ones_d[:])
            nc.gpsimd.dma_start(ident_sb[:], ident_d[:])
            nc.gpsimd.dma_start(
                mdiag_slab[:].rearrange("p (r s) -> p r s", r=NCH),
                mdiag_d.rearrange("(r p) s -> p r s", p=P))

        for rep in range(repeat):
            # ============== phase 1: QKV projection + RoPE ==============
            with tc.tile_pool(name="p1", bufs=1) as p1, \
                 tc.tile_pool(name="xin", bufs=6) as xin, \
                 tc.tile_pool(name="rtmp", bufs=3) as rtmp:

                XQ = NDT // 4   # 4 d-tiles per quarter x slab
                vT = p1.tile([HD, S], FP32, tag="vT")

                def load_wq_head(m):
                    nc.sync.dma_start(
                        wqs[:, m * NDT * HD:(m + 1) * NDT * HD]
                        .rearrange("p (n q) -> p n q", n=NDT),
                        wq_d[:, m * HD:(m + 1) * HD]
                        .rearrange("(n p) q -> p n q", p=P))

                def load_x_slab(xs, qq, s0):
                    nc.sync.dma_start(
                        xs[:].rearrange("p (n s) -> p n s", n=XQ),
                        xT_d[qq * XQ * P:(qq + 1) * XQ * P, s0:s0 + SC]
                        .rearrange("(n p) s -> p n s", p=P))

                if rep == 0:
                    # warm the PE p-state with dummy matmuls on a memset
                    # scratch tile while the first input DMAs are in flight
                    wsrc = p1.tile([P, P], MDT, tag="wsrc")
                    nc.gpsimd.memset(wsrc[:], 0.0)
                    load_wq_head(0)
                    for qq in range(4):
                        load_x_slab(x0_slabs[qq], qq, 0)
                    # weight-slab head-major: head m's d-tile k lives at
                    # columns [m*NDT*HD + k*HD, ...).  One DMA per head so
                    # the first chain is gated by ~2.5 MB only; RoPE tables
                    # next (m=0's RoPE), then the later-phase weights.
                    nc.sync.dma_start(cosT_sb[:], cosT_d[:])
                    nc.sync.dma_start(sinrT_sb[:], sinrT_d[:])
                    for m in range(1, NH):
                        load_wq_head(m)
                    nc.sync.dma_start(
                        wks[:].rearrange("p (n m) -> p n m", n=NDT),
                        wk_d.rearrange("(n p) m -> p n m", p=P))
                    nc.sync.dma_start(
                        wvs[:].rearrange("p (n m) -> p n m", n=NDT),
                        wv_d.rearrange("(n p) m -> p n m", p=P))
                    load_consts()
                    nc.sync.dma_start(
                        wos[:].rearrange("p (n d) -> p n d", n=NH),
                        wo_d.rearrange("(n p) d -> p n d", p=P))
                    for i in range(48):
                        wps = psp.tile([P, SC], FP32, tag="mm", bufs=3)
                        nc.tensor.matmul(wps[:, 0:P], wsrc[:], wsrc[:],
                                         start=True, stop=True)
                else:
                    # x chunk 0 lives in persistent slabs whose only hazard
                    # is the PREVIOUS rep's early phase-1 reads, so this
                    # prefetch drains during the previous rep's phase 2
                    for qq in range(4):
                        load_x_slab(x0_slabs[qq], qq, 0)

                for sc in range(NCH):
                    s0 = sc * SC
                    if sc == 0:
                        xq_slabs = x0_slabs
                    else:
                        xq_slabs = []
                        for qq in range(4):
                            xs = xin.tile([P, XQ * SC], MDT, tag="x")
                            load_x_slab(xs, qq, s0)
                            xq_slabs.append(xs)

                    def xts_k(k):
                        return xq_slabs[k // XQ][:, (k % XQ) * SC:(k % XQ + 1) * SC]

                    # m = 0..3: q heads; 4: k; 5: v
                    for m in range(NH + 2):
                        psum = psp.tile([P, SC], FP32, tag="mm", bufs=3)
                        for k in range(NDT):
                            if m < NH:
                                lhsT = wqs[:, m * NDT * HD + k * HD:
                                           m * NDT * HD + (k + 1) * HD]
                            elif m == NH:
                                lhsT = wks[:, k * HD:(k + 1) * HD]
                            else:
                                lhsT = wvs[:, k * HD:(k + 1) * HD]
                            nc.tensor.matmul(
                                psum[:], lhsT, xts_k(k),
                                start=(k == 0), stop=(k == NDT - 1),
                            )
                        if m <= NH:
                            # RoPE: dst = psum*cosT + shift(psum)*sinrotT
                            dst = (qt[m] if m < NH else kt)[:, s0:s0 + SC]
                            t0 = rtmp.tile([P, SC], FP32, tag="t0")
                            t1 = rtmp.tile([P, SC], FP32, tag="t1")
                            nc.vector.tensor_mul(
                                t0[:], psum[:], cosT_sb[:, s0:s0 + SC])
                            nc.vector.tensor_mul(
                                t1[0:64, :], psum[64:128, :],
                                sinrT_sb[0:64, s0:s0 + SC])
                            nc.vector.tensor_mul(
                                t1[64:128, :], psum[0:64, :],
                                sinrT_sb[64:128, s0:s0 + SC])
                            nc.vector.tensor_add(dst, t0[:], t1[:])
                        else:
                            nc.vector.tensor_copy(vT[:, s0:s0 + SC], psum[:])

                    # transpose this chunk of V^T -> V tiles [S_k=128, HD]
                    for kk in range(SC // P):
                        k = sc * (SC // P) + kk
                        ps_t = psp.tile([P, P], FP32, tag="vy", bufs=2)
                        nc.tensor.transpose(
                            ps_t[:], vT[:, k * P:(k + 1) * P], ident_sb[:])
                        nc.vector.tensor_copy(v_tiles[k][:], ps_t[:])

            # ========== phase 2: attention + output projection ==========
            # The out-projection for chunk c-1 is emitted interleaved into
            # chunk c's attention (one t-group of 16 matmuls after each
            # head) so the PE never waits on the softmax/normalize chains.
            with tc.tile_pool(name="p2", bufs=2) as p2, \
                 tc.tile_pool(name="pt", bufs=24) as ptp, \
                 tc.tile_pool(name="nrm", bufs=4) as nrm, \
                 tc.tile_pool(name="yst", bufs=4) as yst:

                prev_outT = None

                def emit_outproj_tgroup(outT_tiles, c_prev, t, drain=False):
                    q0p = c_prev * SC
                    for dci in range(NCH):
                        d0 = dci * SC
                        y_ps = psp.tile([P, SC], FP32, tag="vy", bufs=2)
                        for h in range(NH):
                            nc.tensor.matmul(
                                y_ps[:],
                                outT_tiles[h][:, t * P:(t + 1) * P],
                                wo_sb[h][:, d0:d0 + SC],
                                start=(h == 0), stop=(h == NH - 1),
                            )
                        y_sb = yst.tile([P, SC], MDT, tag="ysb")
                        # alternate the PSUM->SBUF copy between the two
                        # element-wise engines (Act is exp-saturated) --
                        # except in the end-of-rep drain, where the DVE is
                        # the backlogged engine and Act idles
                        if dci % 2 == 0 and not drain:
                            nc.vector.tensor_copy(y_sb[:], y_ps[:])
                        else:
                            nc.scalar.activation(
                                y_sb[:], y_ps[:],
                                mybir.ActivationFunctionType.Copy)
                        row0 = q0p + t * P
                        nc.gpsimd.dma_start(
                            y_d[row0:row0 + P, d0:d0 + SC], y_sb[:])

                for c in range(NCH):
                    q0 = c * SC
                    nk = 4 * c + 4          # active sk tiles (causal)
                    outT = [p2.tile([P, SC], MDT, tag=f"ot{h}",
                                    name=f"ot{h}") for h in range(NH)]
                    for h in range(NH):
                        ptm = {}             # k -> (pt tile, off)
                        red = []             # pre-reduced tiles for row sums
                        ks = list(range(nk))
                        for k in ks:
                            # diagonal blocks: sk tile k only attends to
                            # sq >= 128k, i.e. chunk columns [off:512); only
                            # the leading 128 columns of that are a partial
                            # (triangular) mask -- the rest is fully allowed.
                            off = max(0, (k - 4 * c) * P)
                            sc_ps = psp.tile([P, SC], FP32, tag="mm", bufs=3)
                            nc.tensor.matmul(
                                sc_ps[:, off:],
                                kt[:, k * P:(k + 1) * P],
                                qt[h][:, q0 + off:q0 + SC],
                                start=True, stop=True,
                            )
                            pt = ptp.tile([P, SC], MDT, tag="pt")
                            if k >= 4 * c:
                                # triangle columns [off:off+128): scale+mask
                                # on DVE then exp; columns beyond are plain
                                r = k % NCH
                                nc.vector.scalar_tensor_tensor(
                                    sc_ps[:, off:off + P], sc_ps[:, off:off + P],
                                    SCALE, mdiag_sb[r][:, off:off + P],
                                    op0=mybir.AluOpType.mult,
                                    op1=mybir.AluOpType.add)
                                nc.scalar.activation(
                                    pt[:, off:off + P], sc_ps[:, off:off + P],
                                    mybir.ActivationFunctionType.Exp)
                                if off + P < SC:
                                    nc.scalar.activation(
                                        pt[:, off + P:], sc_ps[:, off + P:],
                                        mybir.ActivationFunctionType.Exp,
                                        scale=SCALE)
                            else:
                                nc.scalar.activation(
                                    pt[:, off:], sc_ps[:, off:],
                                    mybir.ActivationFunctionType.Exp,
                                    scale=SCALE)
                            ptm[k] = (pt, off)
                            if k == 4 * c + 3:
                                # staircase-sum the 4 diagonal prob tiles
                                d0 = ptm[4 * c][0]
                                d1 = ptm[4 * c + 1][0]
                                d2 = ptm[4 * c + 2][0]
                                d3 = ptm[4 * c + 3][0]
                                ds = ptp.tile([P, SC], MDT, tag="ds", bufs=4)
                                nc.vector.tensor_copy(ds[:, 0:P], d0[:, 0:P])
                                nc.vector.tensor_add(
                                    ds[:, P:], d0[:, P:], d1[:, P:])
                                nc.vector.tensor_add(
                                    ds[:, 2 * P:], ds[:, 2 * P:], d2[:, 2 * P:])
                                nc.vector.tensor_add(
                                    ds[:, 3 * P:], ds[:, 3 * P:], d3[:, 3 * P:])
                                red.append(ds)
                            if k % 4 == 3 and k < 4 * c:
                                # quad-reduce 4 full off-diagonal prob tiles
                                # (alternating DVE / gpsimd) so the row-sum
                                # matmul pass only streams nk/4 tiles
                                eng = nc.vector if (k // 4) % 2 == 0 else nc.gpsimd
                                p0 = ptm[k - 3][0]
                                p1_ = ptm[k - 2][0]
                                p2_ = ptm[k - 1][0]
                                p3 = ptm[k][0]
                                qa = ptp.tile([P, SC], MDT, tag="qa", bufs=4)
                                qb = ptp.tile([P, SC], MDT, tag="qb", bufs=4)
                                eng.tensor_add(qa[:], p0[:], p1_[:])
                                eng.tensor_add(qb[:], p2_[:], p3[:])
                                eng.tensor_add(qa[:], qa[:], qb[:])
                                red.append(qa)
                        pts = [ptm[k][0] for k in range(nk)]
                        offs = [ptm[k][1] for k in range(nk)]
                        # fill the exp-dependency window with the previous
                        # chunk's out-projection (pure PE work, no deps)
                        if prev_outT is not None:
                            emit_outproj_tgroup(prev_outT, c - 1, h)
                        # AV: outT_h [HD, sq] = sum_k V_k^T @ probsT_k
                        # (accumulated in ks order = probs completion order)
                        av_ps = psp.tile([P, SC], FP32, tag="av", bufs=2)
                        for i, k in enumerate(ks):
                            nc.tensor.matmul(
                                av_ps[:, offs[k]:], v_tiles[k][:],
                                pts[k][:, offs[k]:],
                                start=(i == 0), stop=(i == nk - 1),
                            )
                        # all-ones stationary -> every psum partition gets
                        # the column sum over sk (broadcast for free).
                        # Emitted after AV so the DVE pre-reduction has the
                        # whole AV pass of cover before the PE needs it.
                        sums_ps = psp.tile([P, SC], FP32, tag="sums", bufs=1)
                        for i, rt in enumerate(red):
                            nc.tensor.matmul(
                                sums_ps[:], ones_sb[:], rt[:],
                                start=(i == 0), stop=(i == len(red) - 1),
                            )
                        # normalize: outT[h] = av * (1/sums)
                        recip = nrm.tile([P, SC], FP32, tag="recip")
                        nc.vector.reciprocal(recip[:], sums_ps[:])
                        nc.vector.tensor_mul(outT[h][:], av_ps[:], recip[:])
                    prev_outT = outT

                # drain: out-projection for the last chunk
                for t in range(SC // P):
                    emit_outproj_tgroup(prev_outT, NCH - 1, t, drain=True)

    nc.compile()
    return nc


def _host_tables():
    inv_freq = 1.0 / (THETA ** (np.arange(0, HD, 2, dtype=np.float32) / HD))
    t = np.arange(S, dtype=np.float32)
    freqs = t[:, None] * inv_freq[None, :]              # [S, HD/2]
    emb = np.concatenate([freqs, freqs], axis=-1)       # [S, HD]
    cos = np.cos(emb).astype(np.float32)
    sin = np.sin(emb).astype(np.float32)
    cosT = np.ascontiguousarray(cos.T)                  # [HD, S]
    sinT = np.ascontiguousarray(sin.T)
    sinrotT = sinT.copy()
    sinrotT[0:HD // 2] = -sinT[0:HD // 2]
    return cosT, sinrotT


def get_program(mm_mode=MM_MODE, repeat=1):
    key = ("nc", mm_mode, repeat)
    if key not in _CACHE:
        _CACHE[key] = _build_program(mm_mode, repeat)
    return _CACHE[key]


def _mdt_np(mm_mode):
    return ml_dtypes.bfloat16 if mm_mode == "bf16" else np.float32


def make_in_maps(x, wq, wk, wv, wo, mask, mm_mode=MM_MODE):
    mdt = _mdt_np(mm_mode)
    x = np.asarray(x, dtype=np.float32)
    wq = np.asarray(wq, dtype=np.float32).astype(mdt)
    wk = np.asarray(wk, dtype=np.float32).astype(mdt)
    wv = np.asarray(wv, dtype=np.float32).astype(mdt)
    wo = np.asarray(wo, dtype=np.float32).astype(mdt)
    mask = np.asarray(mask, dtype=np.float32)

    cosT, sinrotT = _host_tables()
    ident = np.eye(P, dtype=np.float32)
    # maskdiag[r*128+a, b] = mask[0,0, b, r*128+a]; pattern repeats per chunk
    maskdiag = np.ascontiguousarray(mask[0, 0, 0:SC, 0:SC].T)

    xT = [np.ascontiguousarray(x[b].T).astype(mdt) for b in range(B)]
    in_maps = []
    for c in range(NCORES):
        b, g = c // NG, c % NG
        qc0 = g * NH * HD
        kc0 = g * HD
        in_maps.append({
            "xT": xT[b],
            "wqg": np.ascontiguousarray(wq[:, qc0:qc0 + NH * HD]),
            "wkg": np.ascontiguousarray(wk[:, kc0:kc0 + HD]),
            "wvg": np.ascontiguousarray(wv[:, kc0:kc0 + HD]),
            "wog": np.ascontiguousarray(wo[qc0:qc0 + NH * HD, :]),
            "cosT": cosT,
            "sinrotT": sinrotT,
            "maskdiag": maskdiag,
            "ident": ident,
            "ones": np.ones((P, P), dtype=np.float32).astype(mdt),
        })
    return in_maps


LAST_RESULTS = None


def _make_exec(nc):
    """Mirror run_bass_via_pjrt's multi-core path, but keep the jitted
    executable so repeated (timed) dispatches skip retrace/reload."""
    import jax
    from jax.experimental.shard_map import shard_map
    from jax.sharding import Mesh, PartitionSpec

    from concourse import bass2jax, mybir as _mybir

    bass2jax.install_neuronx_cc_hook()
    partition_name = (
        nc.partition_id_tensor.name if nc.partition_id_tensor else None)
    in_names, out_names, out_avals, zero_outs = [], [], [], []
    for alloc in nc.m.functions[0].allocations:
        if not isinstance(alloc, _mybir.MemoryLocationSet):
            continue
        name = alloc.memorylocations[0].name
        if alloc.kind == "ExternalInput":
            if name != partition_name:
                in_names.append(name)
        elif alloc.kind == "ExternalOutput":
            shape = tuple(alloc.tensor_shape)
            dtype = _mybir.dt.np(alloc.dtype)
            out_names.append(name)
            out_avals.append(jax.core.ShapedArray(shape, dtype))
            zero_outs.append(np.zeros(shape, dtype))
    n_params = len(in_names)
    n_outs = len(out_avals)
    all_in_names = list(in_names) + list(out_names)
    if partition_name is not None:
        all_in_names.append(partition_name)
    donate = tuple(range(n_params, n_params + n_outs))

    def _body(*args):
        operands = list(args)
        if partition_name is not None:
            operands.append(bass2jax.partition_id_tensor())
        outs = bass2jax._bass_exec_p.bind(
            *operands,
            out_avals=tuple(out_avals),
            in_names=tuple(all_in_names),
            out_names=tuple(out_names),
            lowering_input_output_aliases=(),
            sim_require_finite=True,
            sim_require_nnan=True,
            nc=nc,
        )
        return tuple(outs)

    devices = jax.devices()[:NCORES]
    mesh = Mesh(np.asarray(devices), ("core",))
    sharded = jax.jit(
        shard_map(
            _body, mesh=mesh,
            in_specs=(PartitionSpec("core"),) * (n_params + n_outs),
            out_specs=(PartitionSpec("core"),) * n_outs,
            check_rep=False,
        ),
        donate_argnums=donate, keep_unused=True,
    )
    return {
        "fn": sharded, "in_names": in_names, "out_names": out_names,
        "out_avals": out_avals, "zero_outs": zero_outs, "mesh": mesh,
    }


def get_exec(mm_mode=MM_MODE, repeat=1):
    key = ("exec", mm_mode, repeat)
    if key not in _CACHE:
        _CACHE[key] = _make_exec(get_program(mm_mode, repeat))
    return _CACHE[key]


def _concat_inputs(ex, in_maps):
    return [
        np.concatenate([np.asarray(in_maps[c][name]) for c in range(NCORES)],
                       axis=0)
        for name in ex["in_names"]
    ]


def _concat_zeros(ex):
    return [
        np.zeros((NCORES * z.shape[0], *z.shape[1:]), z.dtype)
        for z in ex["zero_outs"]
    ]


def run_on_device(in_maps, mm_mode=MM_MODE, repeat=1):
    """One dispatch; returns per-core output dicts (numpy)."""
    ex = get_exec(mm_mode, repeat)
    out_arrs = ex["fn"](*_concat_inputs(ex, in_maps), *_concat_zeros(ex))
    res = []
    for c in range(NCORES):
        res.append({
            name: np.asarray(out_arrs[i]).reshape(
                NCORES, *ex["out_avals"][i].shape)[c]
            for i, name in enumerate(ex["out_names"])
        })
    return res


def bench(in_maps, iters=5, mm_mode=MM_MODE, repeat=1):
    """Timed repeated dispatch: inputs pre-placed on device, fresh donated
    zero output buffers pre-placed per iteration. Returns list of wall ns."""
    import time

    import jax
    from jax.sharding import NamedSharding, PartitionSpec

    ex = get_exec(mm_mode, repeat)
    sh = NamedSharding(ex["mesh"], PartitionSpec("core"))
    dev_in = [jax.device_put(a, sh) for a in _concat_inputs(ex, in_maps)]
    zsets = [[jax.device_put(z, sh) for z in _concat_zeros(ex)]
             for _ in range(iters + 1)]
    jax.block_until_ready(dev_in)
    jax.block_until_ready(zsets)
    out = ex["fn"](*dev_in, *zsets[0])       # warm-up
    jax.block_until_ready(out)
    times = []
    for i in range(iters):
        t0 = time.perf_counter()
        out = ex["fn"](*dev_in, *zsets[i + 1])
        jax.block_until_ready(out)
        times.append((time.perf_counter() - t0) * 1e9)
    return times


def bench_slope(in_maps, iters=8, mm_mode=MM_MODE, r_hi=4):
    """Per-iteration kernel time via slope: (T(r_hi) - T(1)) / (r_hi - 1),
    immune to constant dispatch overhead.

    Two noise sources dominate the axon dispatch wall: slow drift of the
    ~70-90 ms overhead, and an executable-switch cost paid by the first
    dispatch after changing NEFFs (size-dependent, so it biases the slope).
    So: run same-executable BATCHES, alternate batches between the two
    executables (cancels drift at batch granularity), drop the first
    dispatch of every batch (absorbs the switch cost), and take the slope
    of the medians of the surviving samples.
    """
    import time

    import jax
    from jax.sharding import NamedSharding, PartitionSpec

    def prep(ex):
        sh = NamedSharding(ex["mesh"], PartitionSpec("core"))
        dev_in = [jax.device_put(a, sh) for a in _concat_inputs(ex, in_maps)]
        zsets = [[jax.device_put(z, sh) for z in _concat_zeros(ex)]
                 for _ in range(iters + 4)]
        jax.block_until_ready(dev_in)
        jax.block_until_ready(zsets)
        return [ex, dev_in, zsets, 0, []]

    s1 = prep(get_exec(mm_mode, 1))
    sh_ = prep(get_exec(mm_mode, r_hi))
    # warm-up both executables once
    for s in (s1, sh_):
        out = s[0]["fn"](*s[1], *s[2][s[3]])
        jax.block_until_ready(out)
        s[3] += 1

    nbatch = 3
    bs = max(2, iters // nbatch)
    for b in range(nbatch):
        for s in (s1, sh_):
            ex, dev_in, zsets, zi, store = s
            for j in range(bs + 1):
                if zi >= len(zsets):
                    break
                t0 = time.perf_counter()
                out = ex["fn"](*dev_in, *zsets[zi])
                jax.block_until_ready(out)
                dt = (time.perf_counter() - t0) * 1e9
                s[3] = zi = zi + 1
                if j > 0:      # first dispatch pays the NEFF switch
                    store.append(dt)

    t1s, ths = s1[4], sh_[4]
    slope = (np.median(ths) - np.median(t1s)) / (r_hi - 1)
    return {
        "t1": t1s, "th": ths,
        "exec_ns_median": float(slope),
        "exec_ns_min": float(slope),
    }


def kernel(x, wq, wk, wv, wo, mask):
    """Full inputs in, full output out; shards over the 8 NeuronCores."""
    global LAST_RESULTS
    from concourse import bass_utils

    nc = get_program()
    in_maps = make_in_maps(x, wq, wk, wv, wo, mask)
    res = bass_utils.run_bass_kernel_spmd(
        nc, in_maps, core_ids=list(range(NCORES)))
    LAST_RESULTS = res
    out = np.zeros((B, S, D), dtype=np.float32)
    for c in range(NCORES):
        b = c // NG
        out[b] += np.asarray(res.results[c]["y"]).astype(np.float32)
    return out


# revision 41
# speedup vs baseline: 1.3602x; 1.2956x over previous
"""Trainium2 Bass kernel for GQA attention (B=2,S=2048,D=2048,H=16,KV=4,HD=128)
with RoPE + causal mask, sharded over 8 NeuronCores:
  2-way data parallel over batch x 4-way tensor parallel over KV groups.

Core c = (b, g): b = c // 4, g = c % 4.
Each core computes, for its batch b and KV group g (q heads 4g..4g+3):
  QT_h [HD,S], KT [HD,S] (RoPE'd), V [S,HD]    via matmul vs xT [D,S]
  scoresT [sk,sq] blocks, exp on ScalarE (scale folded), row-sums via an
  all-ones matmul (which also replicates the sums across partitions),
  AV with V tiles stationary -> outT [HD,sq], per-head normalization via
  reciprocal, partial y = attn_norm @ wo_rows[g]; host sums the 4 partials.

matmul(out, lhsT, rhs) = lhsT.T @ rhs, contraction over the partition dim.
All contractions are K=128.  Causality at block granularity: fully-masked
(sk,sq) blocks skipped; diagonal blocks add the mask slice (pattern repeats
every 4 sk-tiles, so only a [512,512] mask transpose is shipped).

Matmul operands are bf16 (full-kernel relative error ~3e-3 vs the fp32
reference; the gate is 2e-2).  All accumulation is fp32 in PSUM; softmax
exp/normalization arithmetic is fp32.  The output projection for sq-chunk c
is software-pipelined one chunk behind attention so its PE work fills the
softmax-dependency bubbles of chunk c+1, and y tiles are DMA'd straight
from PSUM.
"""

import os
from contextlib import ExitStack

import numpy as np
import ml_dtypes

import concourse.bacc as bacc
import concourse.mybir as mybir
import concourse.tile as tile

# ---------------- problem constants (hardcoded per contract) ----------------
B, S, D = 2, 2048, 2048
H, KV, HD = 16, 4, 128
REP = H // KV            # 4 q heads per kv head
NG = KV                  # 4 tensor-parallel groups
NCORES = 8
THETA = 10000.0
SCALE = 1.0 / float(np.sqrt(HD))

P = 128                  # partition dim
SC = 512                 # moving free-dim chunk (one PSUM bank in fp32)
NDT = S // P             # 16 tiles of 128 along S or D
NCH = S // SC            # 4 chunks of 512 along S
NH = REP                 # 4 q-heads per core

FP32 = mybir.dt.float32
BF16 = mybir.dt.bfloat16

# matmul dtype: "bf16" (measured full-kernel relative error ~3e-3 vs the
# fp32 reference -- gate is 2e-2) or "fp32" (exact to ~1e-6, 4 cyc/row).
MM_MODE = os.environ.get("KERNEL_MM_MODE", "bf16")
# row-sum strategy: "quad" pre-reduces groups of 4 prob tiles on DVE/gpsimd
# so the ones-matmul streams nk/4 tiles; "pe" streams every prob tile
# through the ones-matmul (more PE cycles, no cross-engine chains).
SUMS_MODE = os.environ.get("KERNEL_SUMS", "quad")

_CACHE = {}


def _build_program(mm_mode=MM_MODE, repeat=1):
    MDT = BF16 if mm_mode == "bf16" else FP32

    nc = bacc.Bacc("TRN2", target_bir_lowering=False, debug=False)

    xT_d = nc.dram_tensor("xT", [D, S], MDT, kind="ExternalInput").ap()
    wq_d = nc.dram_tensor("wqg", [D, NH * HD], MDT, kind="ExternalInput").ap()
    wk_d = nc.dram_tensor("wkg", [D, HD], MDT, kind="ExternalInput").ap()
    wv_d = nc.dram_tensor("wvg", [D, HD], MDT, kind="ExternalInput").ap()
    wo_d = nc.dram_tensor("wog", [NH * HD, D], MDT, kind="ExternalInput").ap()
    cosT_d = nc.dram_tensor("cosT", [HD, S], FP32, kind="ExternalInput").ap()
    sinrT_d = nc.dram_tensor("sinrotT", [HD, S], FP32, kind="ExternalInput").ap()
    mdiag_d = nc.dram_tensor("maskdiag", [NCH * P, SC], FP32, kind="ExternalInput").ap()
    ident_d = nc.dram_tensor("ident", [P, P], FP32, kind="ExternalInput").ap()
    ones_d = nc.dram_tensor("ones", [P, P], MDT, kind="ExternalInput").ap()
    y_d = nc.dram_tensor("y", [S, D], MDT, kind="ExternalOutput").ap()

    with tile.TileContext(nc) as tc, ExitStack() as ctx:
        qkv = ctx.enter_context(tc.tile_pool(name="qkv", bufs=1))
        small = ctx.enter_context(tc.tile_pool(name="small", bufs=1))
        # One persistent PSUM pool for the whole program: per-rep pool
        # open/close emits engine drains that serialize rep boundaries.
        # 8 banks: mm(proj+scores) 3, av 2, vy(V-transpose+y-out) 2, sums 1.
        psp = ctx.enter_context(tc.tile_pool(name="psp", bufs=1, space="PSUM"))

        # resident Q^T per head, K^T, V tiles
        qt = [qkv.tile([P, S], MDT, tag=f"qt{h}", name=f"qt{h}") for h in range(NH)]
        kt = qkv.tile([P, S], MDT, tag="kt", name="kt")
        v_tiles = [qkv.tile([P, HD], MDT, tag=f"v{k}", name=f"v{k}")
                   for k in range(NDT)]

        ones_sb = small.tile([P, P], MDT, tag="ones")
        ident_sb = small.tile([P, P], FP32, tag="ident")
        mdiag_slab = small.tile([P, NCH * SC], FP32, tag="mds")
        mdiag_sb = [mdiag_slab[:, r * SC:(r + 1) * SC] for r in range(NCH)]
        wos = small.tile([P, NH * D], MDT, tag="wos")
        wo_sb = [wos[:, h * D:(h + 1) * D] for h in range(NH)]

        # weights + tables are rep-invariant: persistent tiles, loaded once
        # (all on the sync queue -- it idles during phase 2, so nothing
        # blocks at a rep boundary)
        wqs = small.tile([P, NH * NDT * HD], MDT, tag="wqs")
        wks = small.tile([P, NDT * HD], MDT, tag="wks")
        wvs = small.tile([P, NDT * HD], MDT, tag="wvs")
        cosT_sb = small.tile([HD, S], FP32, tag="cosT")
        sinrT_sb = small.tile([HD, S], FP32, tag="sinrT")
        # x chunk 0 is persistent too: its reload for rep r+1 only waits on
        # rep r's early phase-1 reads, never on the phase-2 pool teardown
        x0_slabs = [small.tile([P, (NDT // 4) * SC], MDT, tag=f"x0q{qq}",
                               name=f"x0q{qq}") for qq in range(4)]

        def load_consts():
            nc.gpsimd.dma_start(ones_sb[:], ones_d[:])
            nc.gpsimd.dma_start(ident_sb[:], ident_d[:])
            nc.gpsimd.dma_start(
                mdiag_slab[:].rearrange("p (r s) -> p r s", r=NCH),
                mdiag_d.rearrange("(r p) s -> p r s", p=P))

        for rep in range(repeat):
            # ============== phase 1: QKV projection + RoPE ==============
            with tc.tile_pool(name="p1", bufs=1) as p1, \
                 tc.tile_pool(name="xin", bufs=12) as xin, \
                 tc.tile_pool(name="rtmp", bufs=3) as rtmp:

                XQ = NDT // 4   # 4 d-tiles per quarter x slab
                vT = p1.tile([HD, S], FP32, tag="vT")

                def load_wq_head(m):
                    nc.sync.dma_start(
                        wqs[:, m * NDT * HD:(m + 1) * NDT * HD]
                        .rearrange("p (n q) -> p n q", n=NDT),
                        wq_d[:, m * HD:(m + 1) * HD]
                        .rearrange("(n p) q -> p n q", p=P))

                def load_x_slab(xs, qq, s0):
                    nc.sync.dma_start(
                        xs[:].rearrange("p (n s) -> p n s", n=XQ),
                        xT_d[qq * XQ * P:(qq + 1) * XQ * P, s0:s0 + SC]
                        .rearrange("(n p) s -> p n s", p=P))

                if rep == 0:
                    # warm the PE p-state with dummy matmuls on a memset
                    # scratch tile while the first input DMAs are in flight
                    wsrc = p1.tile([P, P], MDT, tag="wsrc")
                    nc.gpsimd.memset(wsrc[:], 0.0)
                    load_wq_head(0)
                    for qq in range(4):
                        load_x_slab(x0_slabs[qq], qq, 0)
                    # weight-slab head-major: head m's d-tile k lives at
                    # columns [m*NDT*HD + k*HD, ...).  One DMA per head so
                    # the first chain is gated by ~2.5 MB only; RoPE tables
                    # next (m=0's RoPE), then the later-phase weights.
                    nc.sync.dma_start(cosT_sb[:], cosT_d[:])
                    nc.sync.dma_start(sinrT_sb[:], sinrT_d[:])
                    for m in range(1, NH):
                        load_wq_head(m)
                    nc.sync.dma_start(
                        wks[:].rearrange("p (n m) -> p n m", n=NDT),
                        wk_d.rearrange("(n p) m -> p n m", p=P))
                    nc.sync.dma_start(
                        wvs[:].rearrange("p (n m) -> p n m", n=NDT),
                        wv_d.rearrange("(n p) m -> p n m", p=P))
                    load_consts()
                    nc.sync.dma_start(
                        wos[:].rearrange("p (n d) -> p n d", n=NH),
                        wo_d.rearrange("(n p) d -> p n d", p=P))
                    for i in range(48):
                        wps = psp.tile([P, SC], FP32, tag="mm", bufs=3)
                        nc.tensor.matmul(wps[:, 0:P], wsrc[:], wsrc[:],
                                         start=True, stop=True)
                else:
                    # x chunk 0 lives in persistent slabs whose only hazard
                    # is the PREVIOUS rep's early phase-1 reads, so this
                    # prefetch drains during the previous rep's phase 2
                    for qq in range(4):
                        load_x_slab(x0_slabs[qq], qq, 0)

                for sc in range(NCH):
                    s0 = sc * SC
                    if sc == 0:
                        xq_slabs = x0_slabs
                    else:
                        xq_slabs = []
                        for qq in range(4):
                            xs = xin.tile([P, XQ * SC], MDT, tag="x")
                            load_x_slab(xs, qq, s0)
                            xq_slabs.append(xs)

                    def xts_k(k):
                        return xq_slabs[k // XQ][:, (k % XQ) * SC:(k % XQ + 1) * SC]

                    # m = 0..3: q heads; 4: k; 5: v
                    for m in range(NH + 2):
                        psum = psp.tile([P, SC], FP32, tag="mm", bufs=3)
                        for k in range(NDT):
                            if m < NH:
                                lhsT = wqs[:, m * NDT * HD + k * HD:
                                           m * NDT * HD + (k + 1) * HD]
                            elif m == NH:
                                lhsT = wks[:, k * HD:(k + 1) * HD]
                            else:
                                lhsT = wvs[:, k * HD:(k + 1) * HD]
                            nc.tensor.matmul(
                                psum[:], lhsT, xts_k(k),
                                start=(k == 0), stop=(k == NDT - 1),
                            )
                        if m <= NH:
                            # RoPE: dst = psum*cosT + shift(psum)*sinrotT
                            dst = (qt[m] if m < NH else kt)[:, s0:s0 + SC]
                            t0 = rtmp.tile([P, SC], FP32, tag="t0")
                            t1 = rtmp.tile([P, SC], FP32, tag="t1")
                            nc.vector.tensor_mul(
                                t0[:], psum[:], cosT_sb[:, s0:s0 + SC])
                            nc.vector.tensor_mul(
                                t1[0:64, :], psum[64:128, :],
                                sinrT_sb[0:64, s0:s0 + SC])
                            nc.vector.tensor_mul(
                                t1[64:128, :], psum[0:64, :],
                                sinrT_sb[64:128, s0:s0 + SC])
                            nc.vector.tensor_add(dst, t0[:], t1[:])
                        else:
                            nc.vector.tensor_copy(vT[:, s0:s0 + SC], psum[:])

                    # transpose this chunk of V^T -> V tiles [S_k=128, HD]
                    for kk in range(SC // P):
                        k = sc * (SC // P) + kk
                        ps_t = psp.tile([P, P], FP32, tag="vy", bufs=2)
                        nc.tensor.transpose(
                            ps_t[:], vT[:, k * P:(k + 1) * P], ident_sb[:])
                        nc.vector.tensor_copy(v_tiles[k][:], ps_t[:])

            # ========== phase 2: attention + output projection ==========
            # The out-projection for chunk c-1 is emitted interleaved into
            # chunk c's attention (one t-group of 16 matmuls after each
            # head) so the PE never waits on the softmax/normalize chains.
            with tc.tile_pool(name="p2", bufs=2) as p2, \
                 tc.tile_pool(name="pt", bufs=24) as ptp, \
                 tc.tile_pool(name="nrm", bufs=4) as nrm, \
                 tc.tile_pool(name="yst", bufs=4) as yst:

                prev_outT = None

                def emit_outproj_tgroup(outT_tiles, c_prev, t, drain=False):
                    q0p = c_prev * SC
                    for dci in range(NCH):
                        d0 = dci * SC
                        y_ps = psp.tile([P, SC], FP32, tag="vy", bufs=2)
                        for h in range(NH):
                            nc.tensor.matmul(
                                y_ps[:],
                                outT_tiles[h][:, t * P:(t + 1) * P],
                                wo_sb[h][:, d0:d0 + SC],
                                start=(h == 0), stop=(h == NH - 1),
                            )
                        y_sb = yst.tile([P, SC], MDT, tag="ysb")
                        # alternate the PSUM->SBUF copy between the two
                        # element-wise engines (Act is exp-saturated) --
                        # except in the end-of-rep drain, where the DVE is
                        # the backlogged engine and Act idles
                        if dci % 2 == 0 and not drain:
                            nc.vector.tensor_copy(y_sb[:], y_ps[:])
                        else:
                            nc.scalar.activation(
                                y_sb[:], y_ps[:],
                                mybir.ActivationFunctionType.Copy)
                        row0 = q0p + t * P
                        nc.gpsimd.dma_start(
                            y_d[row0:row0 + P, d0:d0 + SC], y_sb[:])

                for c in range(NCH):
                    q0 = c * SC
                    nk = 4 * c + 4          # active sk tiles (causal)
                    outT = [p2.tile([P, SC], MDT, tag=f"ot{h}",
                                    name=f"ot{h}") for h in range(NH)]
                    for h in range(NH):
                        ptm = {}             # k -> (pt tile, off)
                        red = []             # pre-reduced tiles for row sums
                        ks = list(range(nk))
                        for k in ks:
                            # diagonal blocks: sk tile k only attends to
                            # sq >= 128k, i.e. chunk columns [off:512); only
                            # the leading 128 columns of that are a partial
                            # (triangular) mask -- the rest is fully allowed.
                            off = max(0, (k - 4 * c) * P)
                            sc_ps = psp.tile([P, SC], FP32, tag="mm", bufs=3)
                            nc.tensor.matmul(
                                sc_ps[:, off:],
                                kt[:, k * P:(k + 1) * P],
                                qt[h][:, q0 + off:q0 + SC],
                                start=True, stop=True,
                            )
                            pt = ptp.tile([P, SC], MDT, tag="pt")
                            if k >= 4 * c:
                                # triangle columns [off:off+128): scale+mask
                                # on DVE then exp; columns beyond are plain
                                r = k % NCH
                                nc.vector.scalar_tensor_tensor(
                                    sc_ps[:, off:off + P], sc_ps[:, off:off + P],
                                    SCALE, mdiag_sb[r][:, off:off + P],
                                    op0=mybir.AluOpType.mult,
                                    op1=mybir.AluOpType.add)
                                nc.scalar.activation(
                                    pt[:, off:off + P], sc_ps[:, off:off + P],
                                    mybir.ActivationFunctionType.Exp)
                                if off + P < SC:
                                    nc.scalar.activation(
                                        pt[:, off + P:], sc_ps[:, off + P:],
                                        mybir.ActivationFunctionType.Exp,
                                        scale=SCALE)
                            else:
                                nc.scalar.activation(
                                    pt[:, off:], sc_ps[:, off:],
                                    mybir.ActivationFunctionType.Exp,
                                    scale=SCALE)
                            ptm[k] = (pt, off)
                            if SUMS_MODE != "quad":
                                continue
                            if k == 4 * c + 3:
                                # staircase-sum the 4 diagonal prob tiles
                                d0 = ptm[4 * c][0]
                                d1 = ptm[4 * c + 1][0]
                                d2 = ptm[4 * c + 2][0]
                                d3 = ptm[4 * c + 3][0]
                                ds = ptp.tile([P, SC], MDT, tag="ds", bufs=4)
                                nc.vector.tensor_copy(ds[:, 0:P], d0[:, 0:P])
                                nc.vector.tensor_add(
                                    ds[:, P:], d0[:, P:], d1[:, P:])
                                nc.vector.tensor_add(
                                    ds[:, 2 * P:], ds[:, 2 * P:], d2[:, 2 * P:])
                                nc.vector.tensor_add(
                                    ds[:, 3 * P:], ds[:, 3 * P:], d3[:, 3 * P:])
                                red.append(ds)
                            if k % 4 == 3 and k < 4 * c:
                                # quad-reduce 4 full off-diagonal prob tiles
                                # (alternating DVE / gpsimd) so the row-sum
                                # matmul pass only streams nk/4 tiles
                                eng = nc.vector if (k // 4) % 2 == 0 else nc.gpsimd
                                p0 = ptm[k - 3][0]
                                p1_ = ptm[k - 2][0]
                                p2_ = ptm[k - 1][0]
                                p3 = ptm[k][0]
                                qa = ptp.tile([P, SC], MDT, tag="qa", bufs=4)
                                qb = ptp.tile([P, SC], MDT, tag="qb", bufs=4)
                                eng.tensor_add(qa[:], p0[:], p1_[:])
                                eng.tensor_add(qb[:], p2_[:], p3[:])
                                eng.tensor_add(qa[:], qa[:], qb[:])
                                red.append(qa)
                        pts = [ptm[k][0] for k in range(nk)]
                        offs = [ptm[k][1] for k in range(nk)]
                        # fill the exp-dependency window with the previous
                        # chunk's out-projection (pure PE work, no deps)
                        if prev_outT is not None:
                            emit_outproj_tgroup(prev_outT, c - 1, h)
                        # AV: outT_h [HD, sq] = sum_k V_k^T @ probsT_k
                        # (accumulated in ks order = probs completion order)
                        av_ps = psp.tile([P, SC], FP32, tag="av", bufs=2)
                        for i, k in enumerate(ks):
                            nc.tensor.matmul(
                                av_ps[:, offs[k]:], v_tiles[k][:],
                                pts[k][:, offs[k]:],
                                start=(i == 0), stop=(i == nk - 1),
                            )
                        # all-ones stationary -> every psum partition gets
                        # the column sum over sk (broadcast for free).
                        # Emitted after AV so the DVE pre-reduction has the
                        # whole AV pass of cover before the PE needs it.
                        sums_ps = psp.tile([P, SC], FP32, tag="sums", bufs=1)
                        if SUMS_MODE == "quad":
                            for i, rt in enumerate(red):
                                nc.tensor.matmul(
                                    sums_ps[:], ones_sb[:], rt[:],
                                    start=(i == 0), stop=(i == len(red) - 1),
                                )
                        else:
                            for i, k in enumerate(ks):
                                nc.tensor.matmul(
                                    sums_ps[:, offs[k]:], ones_sb[:],
                                    pts[k][:, offs[k]:],
                                    start=(i == 0), stop=(i == nk - 1),
                                )
                        # normalize: outT[h] = av * (1/sums)
                        recip = nrm.tile([P, SC], FP32, tag="recip")
                        nc.vector.reciprocal(recip[:], sums_ps[:])
                        nc.vector.tensor_mul(outT[h][:], av_ps[:], recip[:])
                    prev_outT = outT

                # drain: out-projection for the last chunk
                for t in range(SC // P):
                    emit_outproj_tgroup(prev_outT, NCH - 1, t, drain=True)

    nc.compile()
    return nc


def _host_tables():
    inv_freq = 1.0 / (THETA ** (np.arange(0, HD, 2, dtype=np.float32) / HD))
    t = np.arange(S, dtype=np.float32)
    freqs = t[:, None] * inv_freq[None, :]              # [S, HD/2]
    emb = np.concatenate([freqs, freqs], axis=-1)       # [S, HD]
    cos = np.cos(emb).astype(np.float32)
    sin = np.sin(emb).astype(np.float32)
    cosT = np.ascontiguousarray(cos.T)                  # [HD, S]
    sinT = np.ascontiguousarray(sin.T)
    sinrotT = sinT.copy()
    sinrotT[0:HD // 2] = -sinT[0:HD // 2]
    return cosT, sinrotT


def get_program(mm_mode=MM_MODE, repeat=1):
    key = ("nc", mm_mode, repeat, SUMS_MODE)
    if key not in _CACHE:
        _CACHE[key] = _build_program(mm_mode, repeat)
    return _CACHE[key]


def _mdt_np(mm_mode):
    return ml_dtypes.bfloat16 if mm_mode == "bf16" else np.float32


def make_in_maps(x, wq, wk, wv, wo, mask, mm_mode=MM_MODE):
    mdt = _mdt_np(mm_mode)
    x = np.asarray(x, dtype=np.float32)
    wq = np.asarray(wq, dtype=np.float32).astype(mdt)
    wk = np.asarray(wk, dtype=np.float32).astype(mdt)
    wv = np.asarray(wv, dtype=np.float32).astype(mdt)
    wo = np.asarray(wo, dtype=np.float32).astype(mdt)
    mask = np.asarray(mask, dtype=np.float32)

    cosT, sinrotT = _host_tables()
    ident = np.eye(P, dtype=np.float32)
    # maskdiag[r*128+a, b] = mask[0,0, b, r*128+a]; pattern repeats per chunk
    maskdiag = np.ascontiguousarray(mask[0, 0, 0:SC, 0:SC].T)

    xT = [np.ascontiguousarray(x[b].T).astype(mdt) for b in range(B)]
    in_maps = []
    for c in range(NCORES):
        b, g = c // NG, c % NG
        qc0 = g * NH * HD
        kc0 = g * HD
        in_maps.append({
            "xT": xT[b],
            "wqg": np.ascontiguousarray(wq[:, qc0:qc0 + NH * HD]),
            "wkg": np.ascontiguousarray(wk[:, kc0:kc0 + HD]),
            "wvg": np.ascontiguousarray(wv[:, kc0:kc0 + HD]),
            "wog": np.ascontiguousarray(wo[qc0:qc0 + NH * HD, :]),
            "cosT": cosT,
            "sinrotT": sinrotT,
            "maskdiag": maskdiag,
            "ident": ident,
            "ones": np.ones((P, P), dtype=np.float32).astype(mdt),
        })
    return in_maps


LAST_RESULTS = None


def _make_exec(nc):
    """Mirror run_bass_via_pjrt's multi-core path, but keep the jitted
    executable so repeated (timed) dispatches skip retrace/reload."""
    import jax
    from jax.experimental.shard_map import shard_map
    from jax.sharding import Mesh, PartitionSpec

    from concourse import bass2jax, mybir as _mybir

    bass2jax.install_neuronx_cc_hook()
    partition_name = (
        nc.partition_id_tensor.name if nc.partition_id_tensor else None)
    in_names, out_names, out_avals, zero_outs = [], [], [], []
    for alloc in nc.m.functions[0].allocations:
        if not isinstance(alloc, _mybir.MemoryLocationSet):
            continue
        name = alloc.memorylocations[0].name
        if alloc.kind == "ExternalInput":
            if name != partition_name:
                in_names.append(name)
        elif alloc.kind == "ExternalOutput":
            shape = tuple(alloc.tensor_shape)
            dtype = _mybir.dt.np(alloc.dtype)
            out_names.append(name)
            out_avals.append(jax.core.ShapedArray(shape, dtype))
            zero_outs.append(np.zeros(shape, dtype))
    n_params = len(in_names)
    n_outs = len(out_avals)
    all_in_names = list(in_names) + list(out_names)
    if partition_name is not None:
        all_in_names.append(partition_name)
    donate = tuple(range(n_params, n_params + n_outs))

    def _body(*args):
        operands = list(args)
        if partition_name is not None:
            operands.append(bass2jax.partition_id_tensor())
        outs = bass2jax._bass_exec_p.bind(
            *operands,
            out_avals=tuple(out_avals),
            in_names=tuple(all_in_names),
            out_names=tuple(out_names),
            lowering_input_output_aliases=(),
            sim_require_finite=True,
            sim_require_nnan=True,
            nc=nc,
        )
        return tuple(outs)

    devices = jax.devices()[:NCORES]
    mesh = Mesh(np.asarray(devices), ("core",))
    sharded = jax.jit(
        shard_map(
            _body, mesh=mesh,
            in_specs=(PartitionSpec("core"),) * (n_params + n_outs),
            out_specs=(PartitionSpec("core"),) * n_outs,
            check_rep=False,
        ),
        donate_argnums=donate, keep_unused=True,
    )
    return {
        "fn": sharded, "in_names": in_names, "out_names": out_names,
        "out_avals": out_avals, "zero_outs": zero_outs, "mesh": mesh,
    }


def get_exec(mm_mode=MM_MODE, repeat=1):
    key = ("exec", mm_mode, repeat, SUMS_MODE)
    if key not in _CACHE:
        _CACHE[key] = _make_exec(get_program(mm_mode, repeat))
    return _CACHE[key]


def _concat_inputs(ex, in_maps):
    return [
        np.concatenate([np.asarray(in_maps[c][name]) for c in range(NCORES)],
                       axis=0)
        for name in ex["in_names"]
    ]


def _concat_zeros(ex):
    return [
        np.zeros((NCORES * z.shape[0], *z.shape[1:]), z.dtype)
        for z in ex["zero_outs"]
    ]


def run_on_device(in_maps, mm_mode=MM_MODE, repeat=1):
    """One dispatch; returns per-core output dicts (numpy)."""
    ex = get_exec(mm_mode, repeat)
    out_arrs = ex["fn"](*_concat_inputs(ex, in_maps), *_concat_zeros(ex))
    res = []
    for c in range(NCORES):
        res.append({
            name: np.asarray(out_arrs[i]).reshape(
                NCORES, *ex["out_avals"][i].shape)[c]
            for i, name in enumerate(ex["out_names"])
        })
    return res


def bench(in_maps, iters=5, mm_mode=MM_MODE, repeat=1):
    """Timed repeated dispatch: inputs pre-placed on device, fresh donated
    zero output buffers pre-placed per iteration. Returns list of wall ns."""
    import time

    import jax
    from jax.sharding import NamedSharding, PartitionSpec

    ex = get_exec(mm_mode, repeat)
    sh = NamedSharding(ex["mesh"], PartitionSpec("core"))
    dev_in = [jax.device_put(a, sh) for a in _concat_inputs(ex, in_maps)]
    zsets = [[jax.device_put(z, sh) for z in _concat_zeros(ex)]
             for _ in range(iters + 1)]
    jax.block_until_ready(dev_in)
    jax.block_until_ready(zsets)
    out = ex["fn"](*dev_in, *zsets[0])       # warm-up
    jax.block_until_ready(out)
    times = []
    for i in range(iters):
        t0 = time.perf_counter()
        out = ex["fn"](*dev_in, *zsets[i + 1])
        jax.block_until_ready(out)
        times.append((time.perf_counter() - t0) * 1e9)
    return times


def bench_slope(in_maps, iters=8, mm_mode=MM_MODE, r_hi=4):
    """Per-iteration kernel time via slope: (T(r_hi) - T(1)) / (r_hi - 1),
    immune to constant dispatch overhead.

    Two noise sources dominate the axon dispatch wall: slow drift of the
    ~70-90 ms overhead, and an executable-switch cost paid by the first
    dispatch after changing NEFFs (size-dependent, so it biases the slope).
    So: run same-executable BATCHES, alternate batches between the two
    executables (cancels drift at batch granularity), drop the first
    dispatch of every batch (absorbs the switch cost), and take the slope
    of the medians of the surviving samples.
    """
    import time

    import jax
    from jax.sharding import NamedSharding, PartitionSpec

    def prep(ex):
        sh = NamedSharding(ex["mesh"], PartitionSpec("core"))
        dev_in = [jax.device_put(a, sh) for a in _concat_inputs(ex, in_maps)]
        zsets = [[jax.device_put(z, sh) for z in _concat_zeros(ex)]
                 for _ in range(iters + 4)]
        jax.block_until_ready(dev_in)
        jax.block_until_ready(zsets)
        return [ex, dev_in, zsets, 0, []]

    s1 = prep(get_exec(mm_mode, 1))
    sh_ = prep(get_exec(mm_mode, r_hi))
    # warm-up both executables once
    for s in (s1, sh_):
        out = s[0]["fn"](*s[1], *s[2][s[3]])
        jax.block_until_ready(out)
        s[3] += 1

    nbatch = 3
    bs = max(2, iters // nbatch)
    for b in range(nbatch):
        for s in (s1, sh_):
            ex, dev_in, zsets, zi, store = s
            for j in range(bs + 1):
                if zi >= len(zsets):
                    break
                t0 = time.perf_counter()
                out = ex["fn"](*dev_in, *zsets[zi])
                jax.block_until_ready(out)
                dt = (time.perf_counter() - t0) * 1e9
                s[3] = zi = zi + 1
                if j > 0:      # first dispatch pays the NEFF switch
                    store.append(dt)

    t1s, ths = s1[4], sh_[4]
    slope = (np.median(ths) - np.median(t1s)) / (r_hi - 1)
    return {
        "t1": t1s, "th": ths,
        "exec_ns_median": float(slope),
        "exec_ns_min": float(slope),
    }


def kernel(x, wq, wk, wv, wo, mask):
    """Full inputs in, full output out; shards over the 8 NeuronCores."""
    global LAST_RESULTS
    from concourse import bass_utils

    nc = get_program()
    in_maps = make_in_maps(x, wq, wk, wv, wo, mask)
    res = bass_utils.run_bass_kernel_spmd(
        nc, in_maps, core_ids=list(range(NCORES)))
    LAST_RESULTS = res
    out = np.zeros((B, S, D), dtype=np.float32)
    for c in range(NCORES):
        b = c // NG
        out[b] += np.asarray(res.results[c]["y"]).astype(np.float32)
    return out


# revision 51
# speedup vs baseline: 1.6977x; 1.2481x over previous
"""Trainium2 Bass kernel for GQA attention (B=2,S=2048,D=2048,H=16,KV=4,HD=128)
with RoPE + causal mask, sharded over 8 NeuronCores:
  2-way data parallel over batch x 4-way tensor parallel over KV groups.

Core c = (b, g): b = c // 4, g = c % 4.
Each core computes, for its batch b and KV group g (q heads 4g..4g+3):
  QT_h [HD,S], KT [HD,S] (RoPE'd), V [S,HD]    via matmul vs xT [D,S]
  scoresT [sk,sq] blocks, exp on ScalarE (scale folded), row-sums via an
  all-ones matmul (which also replicates the sums across partitions),
  AV with V tiles stationary -> outT [HD,sq], per-head normalization via
  reciprocal, partial y = attn_norm @ wo_rows[g]; host sums the 4 partials.

matmul(out, lhsT, rhs) = lhsT.T @ rhs, contraction over the partition dim.
All contractions are K=128.  Causality at block granularity: fully-masked
(sk,sq) blocks skipped; diagonal blocks add the mask slice (pattern repeats
every 4 sk-tiles, so only a [512,512] mask transpose is shipped).

Matmul operands are bf16 (full-kernel relative error ~3e-3 vs the fp32
reference; the gate is 2e-2).  All accumulation is fp32 in PSUM; softmax
exp/normalization arithmetic is fp32.  The output projection for sq-chunk c
is software-pipelined one chunk behind attention so its PE work fills the
softmax-dependency bubbles of chunk c+1, and y tiles are DMA'd straight
from PSUM.
"""

import os
from contextlib import ExitStack

import numpy as np
import ml_dtypes

import concourse.bacc as bacc
import concourse.mybir as mybir
import concourse.tile as tile

# ---------------- problem constants (hardcoded per contract) ----------------
B, S, D = 2, 2048, 2048
H, KV, HD = 16, 4, 128
REP = H // KV            # 4 q heads per kv head
NG = KV                  # 4 tensor-parallel groups
NCORES = 8
THETA = 10000.0
SCALE = 1.0 / float(np.sqrt(HD))

P = 128                  # partition dim
SC = 512                 # moving free-dim chunk (one PSUM bank in fp32)
NDT = S // P             # 16 tiles of 128 along S or D
NCH = S // SC            # 4 chunks of 512 along S
NH = REP                 # 4 q-heads per core

FP32 = mybir.dt.float32
BF16 = mybir.dt.bfloat16

# matmul dtype: "bf16" (measured full-kernel relative error ~3e-3 vs the
# fp32 reference -- gate is 2e-2) or "fp32" (exact to ~1e-6, 4 cyc/row).
MM_MODE = os.environ.get("KERNEL_MM_MODE", "bf16")
# row-sum strategy: "quad" pre-reduces groups of 4 prob tiles on DVE/gpsimd
# so the ones-matmul streams nk/4 tiles; "pe" streams every prob tile
# through the ones-matmul (more PE cycles, no cross-engine chains).
SUMS_MODE = os.environ.get("KERNEL_SUMS", "quad")

_CACHE = {}


def _build_program(mm_mode=MM_MODE, repeat=1):
    MDT = BF16 if mm_mode == "bf16" else FP32

    nc = bacc.Bacc("TRN2", target_bir_lowering=False, debug=False)

    xT_d = nc.dram_tensor("xT", [D, S], MDT, kind="ExternalInput").ap()
    wq_d = nc.dram_tensor("wqg", [D, NH * HD], MDT, kind="ExternalInput").ap()
    wk_d = nc.dram_tensor("wkg", [D, HD], MDT, kind="ExternalInput").ap()
    wv_d = nc.dram_tensor("wvg", [D, HD], MDT, kind="ExternalInput").ap()
    wo_d = nc.dram_tensor("wog", [NH * HD, D], MDT, kind="ExternalInput").ap()
    cosT_d = nc.dram_tensor("cosT", [HD, S], FP32, kind="ExternalInput").ap()
    sinrT_d = nc.dram_tensor("sinrotT", [HD, S], FP32, kind="ExternalInput").ap()
    mdiag_d = nc.dram_tensor("maskdiag", [NCH * P, SC], FP32, kind="ExternalInput").ap()
    ident_d = nc.dram_tensor("ident", [P, P], FP32, kind="ExternalInput").ap()
    ones_d = nc.dram_tensor("ones", [P, P], MDT, kind="ExternalInput").ap()
    y_d = nc.dram_tensor("y", [S, D], MDT, kind="ExternalOutput").ap()

    with tile.TileContext(nc) as tc, ExitStack() as ctx:
        qkv = ctx.enter_context(tc.tile_pool(name="qkv", bufs=1))
        small = ctx.enter_context(tc.tile_pool(name="small", bufs=1))
        # One persistent PSUM pool for the whole program: per-rep pool
        # open/close emits engine drains that serialize rep boundaries.
        # 8 banks: mm(proj+scores) 3, av 2, vy(V-transpose+y-out) 2, sums 1.
        psp = ctx.enter_context(tc.tile_pool(name="psp", bufs=1, space="PSUM"))

        # resident Q^T per head, K^T, V tiles
        qt = [qkv.tile([P, S], MDT, tag=f"qt{h}", name=f"qt{h}") for h in range(NH)]
        kt = qkv.tile([P, S], MDT, tag="kt", name="kt")
        v_tiles = [qkv.tile([P, HD], MDT, tag=f"v{k}", name=f"v{k}")
                   for k in range(NDT)]

        ones_sb = small.tile([P, P], MDT, tag="ones")
        ident_sb = small.tile([P, P], FP32, tag="ident")
        mdiag_slab = small.tile([P, NCH * SC], FP32, tag="mds")
        mdiag_sb = [mdiag_slab[:, r * SC:(r + 1) * SC] for r in range(NCH)]
        wos = small.tile([P, NH * D], MDT, tag="wos")
        wo_sb = [wos[:, h * D:(h + 1) * D] for h in range(NH)]

        # weights + tables are rep-invariant: persistent tiles, loaded once
        # (all on the sync queue -- it idles during phase 2, so nothing
        # blocks at a rep boundary)
        wqs = small.tile([P, NH * NDT * HD], MDT, tag="wqs")
        wks = small.tile([P, NDT * HD], MDT, tag="wks")
        wvs = small.tile([P, NDT * HD], MDT, tag="wvs")
        cosT_sb = small.tile([HD, S], FP32, tag="cosT")
        sinrT_sb = small.tile([HD, S], FP32, tag="sinrT")
        # x chunk 0 is persistent too: its reload for rep r+1 only waits on
        # rep r's early phase-1 reads, never on the phase-2 pool teardown
        x0_slabs = [small.tile([P, (NDT // 4) * SC], MDT, tag=f"x0q{qq}",
                               name=f"x0q{qq}") for qq in range(4)]

        def load_consts():
            # NOTE: must be emitted before any reader -- a read emitted
            # before the writer DMA gets no dependency edge at all.
            nc.gpsimd.dma_start(ones_sb[:], ones_d[:])
            nc.gpsimd.dma_start(
                mdiag_slab[:].rearrange("p (r s) -> p r s", r=NCH),
                mdiag_d.rearrange("(r p) s -> p r s", p=P))

        for rep in range(repeat):
            # ============== phase 1: QKV projection + RoPE ==============
            with tc.tile_pool(name="p1", bufs=1) as p1, \
                 tc.tile_pool(name="xin", bufs=12) as xin, \
                 tc.tile_pool(name="rtmp", bufs=3) as rtmp:

                XQ = NDT // 4   # 4 d-tiles per quarter x slab
                vT = p1.tile([HD, S], FP32, tag="vT")

                def load_wq_head(m):
                    nc.sync.dma_start(
                        wqs[:, m * NDT * HD:(m + 1) * NDT * HD]
                        .rearrange("p (n q) -> p n q", n=NDT),
                        wq_d[:, m * HD:(m + 1) * HD]
                        .rearrange("(n p) q -> p n q", p=P))

                def load_x_slab(xs, qq, s0):
                    nc.sync.dma_start(
                        xs[:].rearrange("p (n s) -> p n s", n=XQ),
                        xT_d[qq * XQ * P:(qq + 1) * XQ * P, s0:s0 + SC]
                        .rearrange("(n p) s -> p n s", p=P))

                if rep == 0:
                    # warm the PE p-state with dummy matmuls on a memset
                    # scratch tile while the first input DMAs are in flight
                    wsrc = p1.tile([P, P], MDT, tag="wsrc")
                    nc.gpsimd.memset(wsrc[:], 0.0)
                    # ident is read by the V-transposes from chunk 0 on
                    nc.gpsimd.dma_start(ident_sb[:], ident_d[:])
                    # weight-slab head-major: head m's d-tile k lives at
                    # columns [m*NDT*HD + k*HD, ...).  One DMA per head so
                    # the first chain is gated by ~2.5 MB only; RoPE tables
                    # next (m=0's RoPE).  The remaining weights are issued
                    # interleaved between the x-chunk loads (in the sc loop)
                    # by need-time, so x never queues behind cold weights.
                    load_wq_head(0)
                    load_x_slab(x0_slabs[0], 0, 0)
                    load_x_slab(x0_slabs[1], 1, 0)
                    load_wq_head(1)
                    load_x_slab(x0_slabs[2], 2, 0)
                    load_wq_head(2)
                    load_x_slab(x0_slabs[3], 3, 0)
                    load_wq_head(3)
                    nc.sync.dma_start(cosT_sb[:], cosT_d[:])
                    nc.sync.dma_start(sinrT_sb[:], sinrT_d[:])
                    nc.sync.dma_start(
                        wks[:].rearrange("p (n m) -> p n m", n=NDT),
                        wk_d.rearrange("(n p) m -> p n m", p=P))
                    nc.sync.dma_start(
                        wvs[:].rearrange("p (n m) -> p n m", n=NDT),
                        wv_d.rearrange("(n p) m -> p n m", p=P))
                    for i in range(48):
                        wps = psp.tile([P, SC], FP32, tag="mm", bufs=3)
                        nc.tensor.matmul(wps[:, 0:P], wsrc[:], wsrc[:],
                                         start=True, stop=True)
                else:
                    # x chunk 0 lives in persistent slabs whose only hazard
                    # is the PREVIOUS rep's early phase-1 reads, so this
                    # prefetch drains during the previous rep's phase 2
                    for qq in range(4):
                        load_x_slab(x0_slabs[qq], qq, 0)

                for sc in range(NCH):
                    s0 = sc * SC
                    if sc == 0:
                        xq_slabs = x0_slabs
                    else:
                        xq_slabs = []
                        for qq in range(4):
                            xs = xin.tile([P, XQ * SC], MDT, tag="x")
                            load_x_slab(xs, qq, s0)
                            xq_slabs.append(xs)
                    if rep == 0 and sc == 2:
                        load_consts()
                        nc.sync.dma_start(
                            wos[:].rearrange("p (n d) -> p n d", n=NH),
                            wo_d.rearrange("(n p) d -> p n d", p=P))

                    def xts_k(k):
                        return xq_slabs[k // XQ][:, (k % XQ) * SC:(k % XQ + 1) * SC]

                    # m = 0..3: q heads; 4: k; 5: v
                    for m in range(NH + 2):
                        psum = psp.tile([P, SC], FP32, tag="mm", bufs=3)
                        for k in range(NDT):
                            if m < NH:
                                lhsT = wqs[:, m * NDT * HD + k * HD:
                                           m * NDT * HD + (k + 1) * HD]
                            elif m == NH:
                                lhsT = wks[:, k * HD:(k + 1) * HD]
                            else:
                                lhsT = wvs[:, k * HD:(k + 1) * HD]
                            nc.tensor.matmul(
                                psum[:], lhsT, xts_k(k),
                                start=(k == 0), stop=(k == NDT - 1),
                            )
                        if m <= NH:
                            # RoPE: dst = psum*cosT + shift(psum)*sinrotT
                            dst = (qt[m] if m < NH else kt)[:, s0:s0 + SC]
                            t0 = rtmp.tile([P, SC], FP32, tag="t0")
                            t1 = rtmp.tile([P, SC], FP32, tag="t1")
                            nc.vector.tensor_mul(
                                t0[:], psum[:], cosT_sb[:, s0:s0 + SC])
                            nc.vector.tensor_mul(
                                t1[0:64, :], psum[64:128, :],
                                sinrT_sb[0:64, s0:s0 + SC])
                            nc.vector.tensor_mul(
                                t1[64:128, :], psum[0:64, :],
                                sinrT_sb[64:128, s0:s0 + SC])
                            nc.vector.tensor_add(dst, t0[:], t1[:])
                        else:
                            nc.vector.tensor_copy(vT[:, s0:s0 + SC], psum[:])

                    # transpose this chunk of V^T -> V tiles [S_k=128, HD]
                    for kk in range(SC // P):
                        k = sc * (SC // P) + kk
                        ps_t = psp.tile([P, P], FP32, tag="vy", bufs=2)
                        nc.tensor.transpose(
                            ps_t[:], vT[:, k * P:(k + 1) * P], ident_sb[:])
                        nc.vector.tensor_copy(v_tiles[k][:], ps_t[:])

            # ========== phase 2: attention + output projection ==========
            # The out-projection for chunk c-1 is emitted interleaved into
            # chunk c's attention (one t-group of 16 matmuls after each
            # head) so the PE never waits on the softmax/normalize chains.
            with tc.tile_pool(name="p2", bufs=2) as p2, \
                 tc.tile_pool(name="pt", bufs=24) as ptp, \
                 tc.tile_pool(name="nrm", bufs=4) as nrm, \
                 tc.tile_pool(name="yst", bufs=4) as yst:

                prev_outT = None

                def emit_outproj_tgroup(outT_tiles, c_prev, t, drain=False):
                    q0p = c_prev * SC
                    for dci in range(NCH):
                        d0 = dci * SC
                        y_ps = psp.tile([P, SC], FP32, tag="vy", bufs=2)
                        for h in range(NH):
                            nc.tensor.matmul(
                                y_ps[:],
                                outT_tiles[h][:, t * P:(t + 1) * P],
                                wo_sb[h][:, d0:d0 + SC],
                                start=(h == 0), stop=(h == NH - 1),
                            )
                        y_sb = yst.tile([P, SC], MDT, tag="ysb")
                        # alternate the PSUM->SBUF copy between the two
                        # element-wise engines (Act is exp-saturated) --
                        # except in the end-of-rep drain, where the DVE is
                        # the backlogged engine and Act idles
                        if dci % 2 == 0 and not drain:
                            nc.vector.tensor_copy(y_sb[:], y_ps[:])
                        else:
                            nc.scalar.activation(
                                y_sb[:], y_ps[:],
                                mybir.ActivationFunctionType.Copy)
                        row0 = q0p + t * P
                        nc.gpsimd.dma_start(
                            y_d[row0:row0 + P, d0:d0 + SC], y_sb[:])

                for c in range(NCH):
                    q0 = c * SC
                    nk = 4 * c + 4          # active sk tiles (causal)
                    outT = [p2.tile([P, SC], MDT, tag=f"ot{h}",
                                    name=f"ot{h}") for h in range(NH)]
                    for h in range(NH):
                        ptm = {}             # k -> (pt tile, off)
                        red = []             # pre-reduced tiles for row sums
                        ks = list(range(nk))
                        for k in ks:
                            # diagonal blocks: sk tile k only attends to
                            # sq >= 128k, i.e. chunk columns [off:512); only
                            # the leading 128 columns of that are a partial
                            # (triangular) mask -- the rest is fully allowed.
                            off = max(0, (k - 4 * c) * P)
                            sc_ps = psp.tile([P, SC], FP32, tag="mm", bufs=3)
                            nc.tensor.matmul(
                                sc_ps[:, off:],
                                kt[:, k * P:(k + 1) * P],
                                qt[h][:, q0 + off:q0 + SC],
                                start=True, stop=True,
                            )
                            pt = ptp.tile([P, SC], MDT, tag="pt")
                            if k >= 4 * c:
                                # triangle columns [off:off+128): scale+mask
                                # on DVE then exp; columns beyond are plain
                                r = k % NCH
                                nc.vector.scalar_tensor_tensor(
                                    sc_ps[:, off:off + P], sc_ps[:, off:off + P],
                                    SCALE, mdiag_sb[r][:, off:off + P],
                                    op0=mybir.AluOpType.mult,
                                    op1=mybir.AluOpType.add)
                                nc.scalar.activation(
                                    pt[:, off:off + P], sc_ps[:, off:off + P],
                                    mybir.ActivationFunctionType.Exp)
                                if off + P < SC:
                                    nc.scalar.activation(
                                        pt[:, off + P:], sc_ps[:, off + P:],
                                        mybir.ActivationFunctionType.Exp,
                                        scale=SCALE)
                            else:
                                nc.scalar.activation(
                                    pt[:, off:], sc_ps[:, off:],
                                    mybir.ActivationFunctionType.Exp,
                                    scale=SCALE)
                            ptm[k] = (pt, off)
                            if SUMS_MODE != "quad":
                                continue
                            if k == 4 * c + 3:
                                # staircase-sum the 4 diagonal prob tiles
                                d0 = ptm[4 * c][0]
                                d1 = ptm[4 * c + 1][0]
                                d2 = ptm[4 * c + 2][0]
                                d3 = ptm[4 * c + 3][0]
                                ds = ptp.tile([P, SC], MDT, tag="ds", bufs=4)
                                nc.vector.tensor_copy(ds[:, 0:P], d0[:, 0:P])
                                nc.vector.tensor_add(
                                    ds[:, P:], d0[:, P:], d1[:, P:])
                                nc.vector.tensor_add(
                                    ds[:, 2 * P:], ds[:, 2 * P:], d2[:, 2 * P:])
                                nc.vector.tensor_add(
                                    ds[:, 3 * P:], ds[:, 3 * P:], d3[:, 3 * P:])
                                red.append(ds)
                            if k % 4 == 3 and k < 4 * c:
                                # quad-reduce 4 full off-diagonal prob tiles
                                # (alternating DVE / gpsimd) so the row-sum
                                # matmul pass only streams nk/4 tiles
                                eng = nc.vector if (k // 4) % 2 == 0 else nc.gpsimd
                                p0 = ptm[k - 3][0]
                                p1_ = ptm[k - 2][0]
                                p2_ = ptm[k - 1][0]
                                p3 = ptm[k][0]
                                qa = ptp.tile([P, SC], MDT, tag="qa", bufs=4)
                                qb = ptp.tile([P, SC], MDT, tag="qb", bufs=4)
                                eng.tensor_add(qa[:], p0[:], p1_[:])
                                eng.tensor_add(qb[:], p2_[:], p3[:])
                                eng.tensor_add(qa[:], qa[:], qb[:])
                                red.append(qa)
                        pts = [ptm[k][0] for k in range(nk)]
                        offs = [ptm[k][1] for k in range(nk)]
                        # fill the exp-dependency window with the previous
                        # chunk's out-projection (pure PE work, no deps)
                        if prev_outT is not None:
                            emit_outproj_tgroup(prev_outT, c - 1, h)
                        # AV: outT_h [HD, sq] = sum_k V_k^T @ probsT_k
                        # (accumulated in ks order = probs completion order)
                        av_ps = psp.tile([P, SC], FP32, tag="av", bufs=2)
                        for i, k in enumerate(ks):
                            nc.tensor.matmul(
                                av_ps[:, offs[k]:], v_tiles[k][:],
                                pts[k][:, offs[k]:],
                                start=(i == 0), stop=(i == nk - 1),
                            )
                        # all-ones stationary -> every psum partition gets
                        # the column sum over sk (broadcast for free).
                        # Emitted after AV so the DVE pre-reduction has the
                        # whole AV pass of cover before the PE needs it.
                        sums_ps = psp.tile([P, SC], FP32, tag="sums", bufs=1)
                        if SUMS_MODE == "quad":
                            for i, rt in enumerate(red):
                                nc.tensor.matmul(
                                    sums_ps[:], ones_sb[:], rt[:],
                                    start=(i == 0), stop=(i == len(red) - 1),
                                )
                        else:
                            for i, k in enumerate(ks):
                                nc.tensor.matmul(
                                    sums_ps[:, offs[k]:], ones_sb[:],
                                    pts[k][:, offs[k]:],
                                    start=(i == 0), stop=(i == nk - 1),
                                )
                        # normalize: outT[h] = av * (1/sums)
                        recip = nrm.tile([P, SC], FP32, tag="recip")
                        nc.vector.reciprocal(recip[:], sums_ps[:])
                        nc.vector.tensor_mul(outT[h][:], av_ps[:], recip[:])
                    prev_outT = outT

                # drain: out-projection for the last chunk
                for t in range(SC // P):
                    emit_outproj_tgroup(prev_outT, NCH - 1, t, drain=True)

    nc.compile()
    return nc


def _host_tables():
    inv_freq = 1.0 / (THETA ** (np.arange(0, HD, 2, dtype=np.float32) / HD))
    t = np.arange(S, dtype=np.float32)
    freqs = t[:, None] * inv_freq[None, :]              # [S, HD/2]
    emb = np.concatenate([freqs, freqs], axis=-1)       # [S, HD]
    cos = np.cos(emb).astype(np.float32)
    sin = np.sin(emb).astype(np.float32)
    cosT = np.ascontiguousarray(cos.T)                  # [HD, S]
    sinT = np.ascontiguousarray(sin.T)
    sinrotT = sinT.copy()
    sinrotT[0:HD // 2] = -sinT[0:HD // 2]
    return cosT, sinrotT


def get_program(mm_mode=MM_MODE, repeat=1):
    key = ("nc", mm_mode, repeat, SUMS_MODE)
    if key not in _CACHE:
        _CACHE[key] = _build_program(mm_mode, repeat)
    return _CACHE[key]


def _mdt_np(mm_mode):
    return ml_dtypes.bfloat16 if mm_mode == "bf16" else np.float32


def make_in_maps(x, wq, wk, wv, wo, mask, mm_mode=MM_MODE):
    mdt = _mdt_np(mm_mode)
    x = np.asarray(x, dtype=np.float32)
    wq = np.asarray(wq, dtype=np.float32).astype(mdt)
    wk = np.asarray(wk, dtype=np.float32).astype(mdt)
    wv = np.asarray(wv, dtype=np.float32).astype(mdt)
    wo = np.asarray(wo, dtype=np.float32).astype(mdt)
    mask = np.asarray(mask, dtype=np.float32)

    cosT, sinrotT = _host_tables()
    ident = np.eye(P, dtype=np.float32)
    # maskdiag[r*128+a, b] = mask[0,0, b, r*128+a]; pattern repeats per chunk
    maskdiag = np.ascontiguousarray(mask[0, 0, 0:SC, 0:SC].T)

    xT = [np.ascontiguousarray(x[b].T).astype(mdt) for b in range(B)]
    in_maps = []
    for c in range(NCORES):
        b, g = c // NG, c % NG
        qc0 = g * NH * HD
        kc0 = g * HD
        in_maps.append({
            "xT": xT[b],
            "wqg": np.ascontiguousarray(wq[:, qc0:qc0 + NH * HD]),
            "wkg": np.ascontiguousarray(wk[:, kc0:kc0 + HD]),
            "wvg": np.ascontiguousarray(wv[:, kc0:kc0 + HD]),
            "wog": np.ascontiguousarray(wo[qc0:qc0 + NH * HD, :]),
            "cosT": cosT,
            "sinrotT": sinrotT,
            "maskdiag": maskdiag,
            "ident": ident,
            "ones": np.ones((P, P), dtype=np.float32).astype(mdt),
        })
    return in_maps


LAST_RESULTS = None


def _make_exec(nc):
    """Mirror run_bass_via_pjrt's multi-core path, but keep the jitted
    executable so repeated (timed) dispatches skip retrace/reload."""
    import jax
    from jax.experimental.shard_map import shard_map
    from jax.sharding import Mesh, PartitionSpec

    from concourse import bass2jax, mybir as _mybir

    bass2jax.install_neuronx_cc_hook()
    partition_name = (
        nc.partition_id_tensor.name if nc.partition_id_tensor else None)
    in_names, out_names, out_avals, zero_outs = [], [], [], []
    for alloc in nc.m.functions[0].allocations:
        if not isinstance(alloc, _mybir.MemoryLocationSet):
            continue
        name = alloc.memorylocations[0].name
        if alloc.kind == "ExternalInput":
            if name != partition_name:
                in_names.append(name)
        elif alloc.kind == "ExternalOutput":
            shape = tuple(alloc.tensor_shape)
            dtype = _mybir.dt.np(alloc.dtype)
            out_names.append(name)
            out_avals.append(jax.core.ShapedArray(shape, dtype))
            zero_outs.append(np.zeros(shape, dtype))
    n_params = len(in_names)
    n_outs = len(out_avals)
    all_in_names = list(in_names) + list(out_names)
    if partition_name is not None:
        all_in_names.append(partition_name)
    donate = tuple(range(n_params, n_params + n_outs))

    def _body(*args):
        operands = list(args)
        if partition_name is not None:
            operands.append(bass2jax.partition_id_tensor())
        outs = bass2jax._bass_exec_p.bind(
            *operands,
            out_avals=tuple(out_avals),
            in_names=tuple(all_in_names),
            out_names=tuple(out_names),
            lowering_input_output_aliases=(),
            sim_require_finite=True,
            sim_require_nnan=True,
            nc=nc,
        )
        return tuple(outs)

    devices = jax.devices()[:NCORES]
    mesh = Mesh(np.asarray(devices), ("core",))
    sharded = jax.jit(
        shard_map(
            _body, mesh=mesh,
            in_specs=(PartitionSpec("core"),) * (n_params + n_outs),
            out_specs=(PartitionSpec("core"),) * n_outs,
            check_rep=False,
        ),
        donate_argnums=donate, keep_unused=True,
    )
    return {
        "fn": sharded, "in_names": in_names, "out_names": out_names,
        "out_avals": out_avals, "zero_outs": zero_outs, "mesh": mesh,
    }


def get_exec(mm_mode=MM_MODE, repeat=1):
    key = ("exec", mm_mode, repeat, SUMS_MODE)
    if key not in _CACHE:
        _CACHE[key] = _make_exec(get_program(mm_mode, repeat))
    return _CACHE[key]


def _concat_inputs(ex, in_maps):
    return [
        np.concatenate([np.asarray(in_maps[c][name]) for c in range(NCORES)],
                       axis=0)
        for name in ex["in_names"]
    ]


def _concat_zeros(ex):
    return [
        np.zeros((NCORES * z.shape[0], *z.shape[1:]), z.dtype)
        for z in ex["zero_outs"]
    ]


def run_on_device(in_maps, mm_mode=MM_MODE, repeat=1):
    """One dispatch; returns per-core output dicts (numpy)."""
    ex = get_exec(mm_mode, repeat)
    out_arrs = ex["fn"](*_concat_inputs(ex, in_maps), *_concat_zeros(ex))
    res = []
    for c in range(NCORES):
        res.append({
            name: np.asarray(out_arrs[i]).reshape(
                NCORES, *ex["out_avals"][i].shape)[c]
            for i, name in enumerate(ex["out_names"])
        })
    return res


def bench(in_maps, iters=5, mm_mode=MM_MODE, repeat=1):
    """Timed repeated dispatch: inputs pre-placed on device, fresh donated
    zero output buffers pre-placed per iteration. Returns list of wall ns."""
    import time

    import jax
    from jax.sharding import NamedSharding, PartitionSpec

    ex = get_exec(mm_mode, repeat)
    sh = NamedSharding(ex["mesh"], PartitionSpec("core"))
    dev_in = [jax.device_put(a, sh) for a in _concat_inputs(ex, in_maps)]
    zsets = [[jax.device_put(z, sh) for z in _concat_zeros(ex)]
             for _ in range(iters + 1)]
    jax.block_until_ready(dev_in)
    jax.block_until_ready(zsets)
    out = ex["fn"](*dev_in, *zsets[0])       # warm-up
    jax.block_until_ready(out)
    times = []
    for i in range(iters):
        t0 = time.perf_counter()
        out = ex["fn"](*dev_in, *zsets[i + 1])
        jax.block_until_ready(out)
        times.append((time.perf_counter() - t0) * 1e9)
    return times


def bench_slope(in_maps, iters=8, mm_mode=MM_MODE, r_hi=4):
    """Per-iteration kernel time via slope: (T(r_hi) - T(1)) / (r_hi - 1),
    immune to constant dispatch overhead.

    Two noise sources dominate the axon dispatch wall: slow drift of the
    ~70-90 ms overhead, and an executable-switch cost paid by the first
    dispatch after changing NEFFs (size-dependent, so it biases the slope).
    So: run same-executable BATCHES, alternate batches between the two
    executables (cancels drift at batch granularity), drop the first
    dispatch of every batch (absorbs the switch cost), and take the slope
    of the medians of the surviving samples.
    """
    import time

    import jax
    from jax.sharding import NamedSharding, PartitionSpec

    def prep(ex):
        sh = NamedSharding(ex["mesh"], PartitionSpec("core"))
        dev_in = [jax.device_put(a, sh) for a in _concat_inputs(ex, in_maps)]
        zsets = [[jax.device_put(z, sh) for z in _concat_zeros(ex)]
                 for _ in range(iters + 4)]
        jax.block_until_ready(dev_in)
        jax.block_until_ready(zsets)
        return [ex, dev_in, zsets, 0, []]

    s1 = prep(get_exec(mm_mode, 1))
    sh_ = prep(get_exec(mm_mode, r_hi))
    # warm-up both executables once
    for s in (s1, sh_):
        out = s[0]["fn"](*s[1], *s[2][s[3]])
        jax.block_until_ready(out)
        s[3] += 1

    nbatch = 3
    bs = max(2, iters // nbatch)
    for b in range(nbatch):
        for s in (s1, sh_):
            ex, dev_in, zsets, zi, store = s
            for j in range(bs + 1):
                if zi >= len(zsets):
                    break
                t0 = time.perf_counter()
                out = ex["fn"](*dev_in, *zsets[zi])
                jax.block_until_ready(out)
                dt = (time.perf_counter() - t0) * 1e9
                s[3] = zi = zi + 1
                if j > 0:      # first dispatch pays the NEFF switch
                    store.append(dt)

    t1s, ths = s1[4], sh_[4]
    slope = (np.median(ths) - np.median(t1s)) / (r_hi - 1)
    return {
        "t1": t1s, "th": ths,
        "exec_ns_median": float(slope),
        "exec_ns_min": float(slope),
    }


def kernel(x, wq, wk, wv, wo, mask):
    """Full inputs in, full output out; shards over the 8 NeuronCores."""
    global LAST_RESULTS
    from concourse import bass_utils

    nc = get_program()
    in_maps = make_in_maps(x, wq, wk, wv, wo, mask)
    res = bass_utils.run_bass_kernel_spmd(
        nc, in_maps, core_ids=list(range(NCORES)))
    LAST_RESULTS = res
    out = np.zeros((B, S, D), dtype=np.float32)
    for c in range(NCORES):
        b = c // NG
        out[b] += np.asarray(res.results[c]["y"]).astype(np.float32)
    return out
